# revision 42
# baseline (speedup 1.0000x reference)
"""Trainium2 Bass kernel for nn_BiSDA (spiking bi-directional sparse attention).

v2 strategy (8 NeuronCores, single SPMD launch), core c = b*4 + j:
  - Layout: all big per-core tensors are [128, 8192] with free index
    w*1024 + t*256 + s  (window-major, s = dh*16+dw within window, dt=j).
  - v-chain (lif(bn(x))) depends only on x: its BN stats AllReduce runs at
    kernel start (also warming the collective path) and the whole v-LIF
    executes on DVE underneath the conv (PE-bound) phase.
  - Conv psum evacuation runs on ACT (Copy + accum_out sum stats); squares
    run as tensor_tensor_reduce on DVE. No DVE copies.
  - Gather-mean over routed windows: fp8e4 DoubleRow matmuls (two stacked
    128-contractions per instruction, 0.5 cyc/row) with mask-scaled
    0.25-identity lhs; spikes are written directly as fp8 (exact).
  - Attention dot: amat_t lhs columns carry 0.5*2^t so the attention LIF
    runs in scaled form (U_t = 2^t u_t), bit-exact in fp16; the z LIF is
    likewise scaled and exact in bf16.
  - q/k LIF stay fp32: q on DVE, k u/W-updates on Pool (k spike on DVE).
  - Final projection: exact bf16 hi/lo split of pw against binary spikes;
    stats via ACT accum; second AllReduce; affine + streamed output DMA.
"""

import os
import sys

import numpy as np

sys.path.insert(0, "/opt/trn_rl_repo")

import ml_dtypes  # noqa: E402

T, B, C = 4, 2, 128
D, H, W = 8, 32, 32
NUM_WINS = 8
LH, LW = 16, 16
NUM_HEADS, HEAD_DIM = 8, 16
THETA = 0.7
EPS = 1e-5
NTOT = float(T * B * D * H * W)
REGION_N = float(T * 4 * LH * LW)

DEBUG = bool(int(os.environ.get("BISDA_DEBUG", "0")))

_COMPILED = {}


def _build(debug):
    import concourse.bacc as bacc
    import concourse.mybir as mybir
    from concourse import tile

    dt = mybir.dt
    Alu = mybir.AluOpType
    Act = mybir.ActivationFunctionType
    DR = mybir.MatmulPerfMode.DoubleRow

    nc = bacc.Bacc("TRN2", target_bir_lowering=False, debug=False,
                   enable_asserts=False, num_devices=8)

    # ---------------- DRAM I/O ----------------
    xconv = nc.dram_tensor("xconv", [T, 2, 3, 128, 34 * 34], dt.float32r,
                           kind="ExternalInput")
    xv = nc.dram_tensor("xv", [128, NUM_WINS, T, 256], dt.float32,
                        kind="ExternalInput")
    w27 = nc.dram_tensor("w27", [128, 27 * 128], dt.float32r, kind="ExternalInput")
    kwT = nc.dram_tensor("kwT", [128, 128], dt.float32r, kind="ExternalInput")
    pwT2 = nc.dram_tensor("pwT2", [2, 128, 128], dt.bfloat16, kind="ExternalInput")
    # gb columns: q_gamma,q_beta,k_gamma,k_beta,v_gamma,v_beta,p_gamma,p_beta
    gb = nc.dram_tensor("gb", [128, 8], dt.float32, kind="ExternalInput")
    identw = nc.dram_tensor("identw", [128, 128], dt.bfloat16,
                            kind="ExternalInput")  # 0.25 * I
    amats = nc.dram_tensor("amats", [128, 32], dt.bfloat16,
                           kind="ExternalInput")  # col t*8+h: 0.5*2^t one-hot
    emat8 = nc.dram_tensor("emat8", [8, 128], dt.float8e4,
                           kind="ExternalInput")  # 1.0 one-hot expand
    bmask16 = nc.dram_tensor("bmask16", [128, 16], dt.float32,
                             kind="ExternalInput")
    # routing broadcast consts: cols 0:64 wsel[k,(w,w')]= (k==w);
    # 64:128 mask1[k,(w,w')] = (k==w'); 128:256 ones
    romats = nc.dram_tensor("romats", [8, 256], dt.float32,
                            kind="ExternalInput")

    out_d = nc.dram_tensor("out", [T, 2, 128, 1024], dt.bfloat16,
                           kind="ExternalOutput")
    dbg = {}
    if debug:
        dbg["qlin"] = nc.dram_tensor("dbg_qlin", [128, 8192], dt.float32,
                                     kind="ExternalOutput")
        dbg["klin"] = nc.dram_tensor("dbg_klin", [128, 8192], dt.float32,
                                     kind="ExternalOutput")
        dbg["stats"] = nc.dram_tensor("dbg_stats", [128, 40], dt.float32,
                                      kind="ExternalOutput")
        dbg["m"] = nc.dram_tensor("dbg_m", [8, 8], dt.float32,
                                  kind="ExternalOutput")
        dbg["qs"] = nc.dram_tensor("dbg_qs", [128, 8192], dt.float8e4,
                                   kind="ExternalOutput")
        dbg["ks"] = nc.dram_tensor("dbg_ks", [128, 8192], dt.float8e4,
                                   kind="ExternalOutput")
        dbg["vs"] = nc.dram_tensor("dbg_vs", [128, 8192], dt.float8e4,
                                   kind="ExternalOutput")
        dbg["attn"] = nc.dram_tensor("dbg_attn", [8, 8192], dt.float16,
                                     kind="ExternalOutput")
        dbg["z"] = nc.dram_tensor("dbg_z", [4, 128, 2048], dt.bfloat16,
                                  kind="ExternalOutput")
        dbg["p"] = nc.dram_tensor("dbg_p", [128, 8192], dt.bfloat16,
                                  kind="ExternalOutput")
        dbg["vag"] = nc.dram_tensor("dbg_vag", [128, 8192], dt.float8e4,
                                    kind="ExternalOutput")
        dbg["qk"] = nc.dram_tensor("dbg_qk", [128, 512], dt.bfloat16,
                                   kind="ExternalOutput")
        dbg["scid"] = nc.dram_tensor("dbg_scid", [128, 8192], dt.float8e4,
                                     kind="ExternalOutput")
        dbg["scid_e"] = nc.dram_tensor("dbg_scid_e", [128, 8192], dt.float8e4,
                                       kind="ExternalOutput")
        dbg["vag_e"] = nc.dram_tensor("dbg_vag_e", [128, 8, 2, 256],
                                      dt.float8e4, kind="ExternalOutput")
        dbg["attn_e"] = nc.dram_tensor("dbg_attn_e", [8, 4096], dt.float16,
                                       kind="ExternalOutput")

    RG = [[0, 1, 2, 3, 4, 5, 6, 7]]
    P2 = [1.0, 2.0, 4.0, 8.0]  # 2^t

    with tile.TileContext(nc) as tc:
        with (
            tc.tile_pool(name="const", bufs=1) as constp,
            tc.tile_pool(name="dram", bufs=1, space="DRAM") as dramp,
            tc.tile_pool(name="big", bufs=1) as bigp,
            tc.tile_pool(name="small", bufs=1) as smallp,
            tc.tile_pool(name="scr512", bufs=2) as scr512,
        ):
            # ---- constants ----
            kwT_sb = constp.tile([128, 128], dt.float32r, tag="kwT")
            nc.sync.dma_start(kwT_sb[:], kwT[:])
            pwT_sb = constp.tile([128, 256], dt.bfloat16, tag="pwT")
            nc.sync.dma_start(pwT_sb[:, 0:128], pwT2[0])
            nc.sync.dma_start(pwT_sb[:, 128:256], pwT2[1])
            gb_sb = constp.tile([128, 8], dt.float32, tag="gb")
            nc.sync.dma_start(gb_sb[:], gb[:])
            ident_sb = constp.tile([128, 128], dt.bfloat16, tag="ident")
            nc.sync.dma_start(ident_sb[:], identw[:])
            amats_sb = constp.tile([128, 32], dt.bfloat16, tag="amats")
            nc.sync.dma_start(amats_sb[:], amats[:])
            emat_sb = constp.tile([8, 128], dt.float8e4, tag="emat")
            nc.sync.dma_start(emat_sb[:], emat8[:])
            bm16_sb = constp.tile([128, 16], dt.float32, tag="bm16")
            nc.sync.dma_start(bm16_sb[:], bmask16[:])
            ro_sb = constp.tile([8, 256], dt.float32, tag="romats")
            nc.sync.dma_start(ro_sb[:], romats[:])

            # persistent big tensors
            qlin = bigp.tile([128, 8192], dt.float32, tag="qlin")
            klin = bigp.tile([128, 8192], dt.float32, tag="klin")
            vs8 = bigp.tile([128, 8192], dt.float8e4, tag="vs8")

            def wts(big):
                return big[:].rearrange("p (w t s) -> p w t s",
                                        w=8, t=4, s=256)

            def v3(tl):
                return tl[:].rearrange("p (w s) -> p w s", w=8, s=256)

            sums = smallp.tile([128, 72], dt.float32, tag="sums")
            # qsum 0:16, qsq 16:32, ksum 32:48, ksq 48:64, xsum 64:66,
            # xsq 66:68  (q/k slots: (t*2+g)*2+hh)
            qregs = smallp.tile([128, 32], dt.float32, tag="qregs")
            kregs = smallp.tile([128, 32], dt.float32, tag="kregs")
            psums = smallp.tile([128, 16], dt.float32, tag="psums")
            psq = smallp.tile([128, 16], dt.float32, tag="psq")
            aff = smallp.tile([128, 16], dt.float32, tag="aff")
            # aff cols: 0 aq_h,1 bq_h,2 ak_h,3 bk_h,4 av_h,5 bv_h,
            #           6 aq_f,7 bq_f,8 ak_f,9 bk_f

            def bn_affine(scol, gcol, St_tile, dst_half, dst_full, name):
                mean = smallp.tile([128, 1], dt.float32, tag=f"mean{name}")
                nc.vector.tensor_scalar(mean[:], St_tile[:, scol:scol + 1],
                                        1.0 / NTOT, None, op0=Alu.mult)
                var = smallp.tile([128, 1], dt.float32, tag=f"var{name}")
                nc.vector.tensor_scalar(var[:], St_tile[:, scol + 1:scol + 2],
                                        1.0 / NTOT, None, op0=Alu.mult)
                msq = smallp.tile([128, 1], dt.float32, tag=f"msq{name}")
                nc.vector.tensor_tensor(msq[:], mean[:], mean[:], op=Alu.mult)
                nc.vector.tensor_tensor(var[:], var[:], msq[:], op=Alu.subtract)
                nc.vector.tensor_scalar(var[:], var[:], EPS, None, op0=Alu.add)
                nc.scalar.sqrt(var[:], var[:])
                rstd = smallp.tile([128, 1], dt.float32, tag=f"rstd{name}")
                nc.vector.reciprocal(rstd[:], var[:])
                afull = smallp.tile([128, 1], dt.float32, tag=f"af{name}")
                nc.vector.tensor_tensor(afull[:], gb_sb[:, gcol:gcol + 1],
                                        rstd[:], op=Alu.mult)
                bfull = smallp.tile([128, 1], dt.float32, tag=f"bf{name}")
                nc.vector.tensor_tensor(bfull[:], afull[:], mean[:], op=Alu.mult)
                nc.vector.tensor_tensor(bfull[:], gb_sb[:, gcol + 1:gcol + 2],
                                        bfull[:], op=Alu.subtract)
                nc.vector.tensor_scalar(aff[:, dst_half:dst_half + 1], afull[:],
                                        0.5, None, op0=Alu.mult)
                nc.vector.tensor_scalar(aff[:, dst_half + 1:dst_half + 2],
                                        bfull[:], 0.5, None, op0=Alu.mult)
                if dst_full is not None:
                    nc.vector.tensor_copy(aff[:, dst_full:dst_full + 1], afull[:])
                    nc.vector.tensor_copy(aff[:, dst_full + 1:dst_full + 2],
                                          bfull[:])

            # ============ STAGE 0 + A: xv, x-stats, AR_v, conv ============
            with (
                tc.tile_pool(name="xvp", bufs=1) as xvp,
                tc.tile_pool(name="vlif", bufs=3) as vlifp,
                tc.tile_pool(name="xin", bufs=6) as xinp,
                tc.tile_pool(name="psA", bufs=6, space="PSUM") as psA,
                tc.tile_pool(name="psK", bufs=2, space="PSUM") as psK,
                tc.tile_pool(name="w27p", bufs=1) as w27p,
            ):
                w27_sb = w27p.tile([128, 27 * 128], dt.float32r, tag="w27")
                nc.sync.dma_start(w27_sb[:, 0:9 * 128], w27[:, 0:9 * 128])
                xpads0 = {}
                for dd in range(3):
                    for g in range(2):
                        xp = xinp.tile([128, 34 * 34], dt.float32r, tag="xpad")
                        nc.sync.dma_start(xp[:], xconv[0, g, dd])
                        xpads0[(g, dd)] = xp
                nc.sync.dma_start(w27_sb[:, 9 * 128:27 * 128],
                                  w27[:, 9 * 128:27 * 128])
                xv_sb = xvp.tile([128, 8192], dt.float32, tag="xv")
                # x stats: copies land in (not-yet-used) qlin/klin space
                nc.scalar.activation(qlin[:, 0:4096], xv_sb[:, 0:4096],
                                     Act.Copy, accum_out=sums[:, 64:65])
                nc.scalar.activation(qlin[:, 4096:8192], xv_sb[:, 4096:8192],
                                     Act.Copy, accum_out=sums[:, 65:66])
                nc.scalar.activation(klin[:, 0:4096], xv_sb[:, 0:4096],
                                     Act.Square, accum_out=sums[:, 66:67])
                nc.scalar.activation(klin[:, 4096:8192], xv_sb[:, 4096:8192],
                                     Act.Square, accum_out=sums[:, 67:68])

                pay_v = smallp.tile([128, 8], dt.float32, tag="pay_v")
                nc.gpsimd.memset(pay_v[:], 0.0)
                nc.vector.tensor_reduce(
                    pay_v[:, 0:1], sums[:, 64:66], axis=mybir.AxisListType.X,
                    op=Alu.add)
                nc.vector.tensor_reduce(
                    pay_v[:, 1:2], sums[:, 66:68], axis=mybir.AxisListType.X,
                    op=Alu.add)
                ccv_in = dramp.tile([128, 8], dt.float32, tag="ccv_in")
                ccv_out = dramp.tile([128, 8], dt.float32, tag="ccv_out")
                nc.sync.dma_start(ccv_in[:], pay_v[:])
                nc.gpsimd.collective_compute(
                    "AllReduce", Alu.add, replica_groups=RG,
                    ins=[ccv_in.opt()], outs=[ccv_out.opt()])
                Sv = smallp.tile([128, 8], dt.float32, tag="Sv")
                nc.sync.dma_start(Sv[:], ccv_out[:])
                bn_affine(0, 4, Sv, 4, None, "v")

                # ---- conv ----
                for t in range(T):
                    if t == 3:
                        nc.sync.dma_start(
                            xv_sb[:],
                            xv[:].rearrange("p w t s -> p (w t s)"))
                    if t == 0:
                        xpads = xpads0
                    else:
                        xpads = {}
                        for dd in range(3):
                            for g in range(2):
                                xp = xinp.tile([128, 34 * 34], dt.float32r,
                                               tag="xpad")
                                nc.sync.dma_start(xp[:], xconv[t, g, dd])
                                xpads[(g, dd)] = xp
                    pss = {}
                    for g in range(2):
                        for hh in range(2):
                            psx = psA.tile([128, 512], dt.float32, tag="psA")
                            pss[(g, hh)] = psx
                    for tap in range(27):
                        dd, rem = divmod(tap, 9)
                        dhh, dww = divmod(rem, 3)
                        lhs = w27_sb[:, tap * 128:(tap + 1) * 128]
                        for g in range(2):
                            for hh in range(2):
                                rhs = xpads[(g, dd)][:].rearrange(
                                    "p (r c) -> p r c", r=34, c=34
                                )[:, 16 * hh + dhh:16 * hh + dhh + 16,
                                  dww:dww + 32]
                                nc.tensor.matmul(pss[(g, hh)][:], lhs, rhs,
                                                 start=(tap == 0),
                                                 stop=(tap == 26))
                    # k_lin from center planes, rhs (ww, dh, dw)
                    for g in range(2):
                        for hh in range(2):
                            slot = (t * 2 + g) * 2 + hh
                            psk = psK.tile([128, 512], dt.float32, tag="psK")
                            ctr = xpads[(g, 1)][:].rearrange(
                                "p (r c) -> p r c", r=34, c=34)
                            rhs = ctr[:, 16 * hh + 1:16 * hh + 17, 1:33]
                            rhs = rhs.rearrange(
                                "p dh (ww dw) -> p ww dh dw", ww=2, dw=16)
                            nc.tensor.matmul(psk[:], kwT_sb[:], rhs,
                                             start=True, stop=True)
                            w0 = 4 * g + 2 * hh
                            kdst = wts(klin)[:, w0:w0 + 2, t]
                            nc.scalar.activation(
                                kdst,
                                psk[:].rearrange("p (ww s) -> p ww s",
                                                 ww=2, s=256),
                                Act.Copy,
                                accum_out=sums[:, 32 + slot:33 + slot])
                            ksc = scr512.tile([128, 512], dt.float32,
                                              tag="sq512")
                            nc.scalar.activation(
                                ksc[:], psk[:], Act.Square,
                                accum_out=sums[:, 48 + slot:49 + slot])
                        rc = g * 16 + t * 4
                        nc.vector.tensor_reduce(
                            kregs[:, rc:rc + 4],
                            wts(klin)[:, 4 * g:4 * g + 4, t],
                            axis=mybir.AxisListType.X, op=Alu.add)
                    for g in range(2):
                        for hh in range(2):
                            slot = (t * 2 + g) * 2 + hh
                            w0 = 4 * g + 2 * hh
                            dstap = wts(qlin)[:, w0:w0 + 2, t].rearrange(
                                "p ww (dh dw) -> p dh ww dw", dh=16, dw=16)
                            nc.scalar.activation(
                                dstap, pss[(g, hh)][:].rearrange(
                                    "p (dh ww dw) -> p dh ww dw",
                                    ww=2, dh=16, dw=16),
                                Act.Copy,
                                accum_out=sums[:, slot:slot + 1])
                            qsc = scr512.tile([128, 512], dt.float32,
                                              tag="sq512")
                            nc.scalar.activation(
                                qsc[:], pss[(g, hh)][:], Act.Square,
                                accum_out=sums[:, 16 + slot:17 + slot])
                        rc = g * 16 + t * 4
                        nc.vector.tensor_reduce(
                            qregs[:, rc:rc + 4],
                            wts(qlin)[:, 4 * g:4 * g + 4, t],
                            axis=mybir.AxisListType.X, op=Alu.add)

                    if t == 1:
                        wv = None
                        for tv in range(T):
                            yv = vlifp.tile([128, 2048], dt.float32, tag="vu")
                            nc.scalar.activation(
                                v3(yv), wts(xv_sb)[:, :, tv],
                                Act.Identity, bias=aff[:, 5:6],
                                scale=aff[:, 4:5])
                            if tv == 0:
                                u = yv
                            else:
                                un = vlifp.tile([128, 2048], dt.float32,
                                                tag="vu")
                                nc.vector.scalar_tensor_tensor(
                                    un[:], wv[:], 0.5, yv[:],
                                    op0=Alu.mult, op1=Alu.add)
                                u = un
                            nc.vector.tensor_scalar(
                                wts(vs8)[:, :, tv], v3(u), 1.0, None,
                                op0=Alu.is_ge)
                            if tv < T - 1:
                                wn = vlifp.tile([128, 2048], dt.float32,
                                                tag="vu")
                                nc.vector.scalar_tensor_tensor(
                                    wn[:], u[:], 1.0, u[:],
                                    op0=Alu.is_lt, op1=Alu.mult)
                                wv = wn

            # ============ STAGE B: AR1 + affines + routing ============
            from contextlib import ExitStack
            latestack = ExitStack()
            latep = latestack.enter_context(tc.tile_pool(name="late", bufs=1))
            p16g = latep.tile([128, 8192], dt.bfloat16, tag="p16")
            qs8 = latep.tile([128, 8192], dt.float8e4, tag="qs8")
            ks8 = latep.tile([128, 8192], dt.float8e4, tag="ks8")
            scid8 = latep.tile([128, 8192], dt.float8e4, tag="scid8")
            vag8 = latep.tile([128, 8192], dt.float8e4, tag="vag8")

            payload = smallp.tile([128, 40], dt.float32, tag="payload")
            for col, (base, cnt) in enumerate(
                    [(0, 16), (16, 16), (32, 16), (48, 16)]):
                nc.vector.tensor_reduce(
                    payload[:, col:col + 1], sums[:, base:base + cnt],
                    axis=mybir.AxisListType.X, op=Alu.add)
            qreg8 = smallp.tile([128, 8], dt.float32, tag="qreg8")
            nc.vector.tensor_reduce(
                qreg8[:].rearrange("p (g w) -> p g w", g=2, w=4),
                qregs[:].rearrange("p (g t w) -> p g w t", g=2, t=4, w=4),
                axis=mybir.AxisListType.X, op=Alu.add)
            kreg8 = smallp.tile([128, 8], dt.float32, tag="kreg8")
            nc.vector.tensor_reduce(
                kreg8[:].rearrange("p (g w) -> p g w", g=2, w=4),
                kregs[:].rearrange("p (g t w) -> p g w t", g=2, t=4, w=4),
                axis=mybir.AxisListType.X, op=Alu.add)
            nc.vector.tensor_tensor(payload[:, 4:12], qreg8[:],
                                    bm16_sb[:, 0:8], op=Alu.mult)
            nc.vector.tensor_tensor(payload[:, 12:20], qreg8[:],
                                    bm16_sb[:, 8:16], op=Alu.mult)
            nc.vector.tensor_tensor(payload[:, 20:28], kreg8[:],
                                    bm16_sb[:, 0:8], op=Alu.mult)
            nc.vector.tensor_tensor(payload[:, 28:36], kreg8[:],
                                    bm16_sb[:, 8:16], op=Alu.mult)
            nc.gpsimd.memset(payload[:, 36:40], 0.0)

            cc1_in = dramp.tile([128, 40], dt.float32, tag="cc1_in")
            cc1_out = dramp.tile([128, 40], dt.float32, tag="cc1_out")
            nc.sync.dma_start(cc1_in[:], payload[:])
            nc.gpsimd.collective_compute(
                "AllReduce", Alu.add, replica_groups=RG,
                ins=[cc1_in.opt()], outs=[cc1_out.opt()])
            St = smallp.tile([128, 40], dt.float32, tag="St")
            nc.sync.dma_start(St[:], cc1_out[:])
            if debug:
                nc.sync.dma_start(dbg["stats"][:], St[:])

            bn_affine(0, 0, St, 0, 6, "q")
            bn_affine(2, 2, St, 2, 8, "k")
            aff2 = smallp.tile([128, 16], dt.float32, tag="aff2")
            for tt in range(4):
                for (cc, base) in ((0, 0), (2, 8)):
                    nc.vector.tensor_scalar(
                        aff2[:, base + 2 * tt:base + 2 * tt + 2],
                        aff[:, cc:cc + 2], float(2.0 ** tt), None,
                        op0=Alu.mult)

            # routing: region means -> BN -> select b -> a_r -> top4 mask
            qr16 = smallp.tile([128, 16], dt.float32, tag="qr16")
            nc.vector.tensor_scalar(qr16[:], St[:, 4:20], 1.0 / REGION_N, None,
                                    op0=Alu.mult)
            nc.scalar.activation(qr16[:], qr16[:], Act.Identity,
                                 bias=aff[:, 7:8], scale=aff[:, 6:7])
            kr16 = smallp.tile([128, 16], dt.float32, tag="kr16")
            nc.vector.tensor_scalar(kr16[:], St[:, 20:36], 1.0 / REGION_N, None,
                                    op0=Alu.mult)
            nc.scalar.activation(kr16[:], kr16[:], Act.Identity,
                                 bias=aff[:, 9:10], scale=aff[:, 8:9])
            tmp8 = smallp.tile([128, 8], dt.float32, tag="tmp8")
            nc.vector.scalar_tensor_tensor(
                tmp8[:], qr16[:, 0:8], bm16_sb[:, 0:1], qr16[:, 8:16],
                op0=Alu.mult, op1=Alu.bypass)
            nc.vector.scalar_tensor_tensor(
                tmp8[:], qr16[:, 8:16], bm16_sb[:, 8:9], tmp8[:],
                op0=Alu.mult, op1=Alu.add)
            tmpk8 = smallp.tile([128, 8], dt.float32, tag="tmpk8")
            nc.vector.scalar_tensor_tensor(
                tmpk8[:], kr16[:, 0:8], bm16_sb[:, 0:1], kr16[:, 8:16],
                op0=Alu.mult, op1=Alu.bypass)
            nc.vector.scalar_tensor_tensor(
                tmpk8[:], kr16[:, 8:16], bm16_sb[:, 8:9], tmpk8[:],
                op0=Alu.mult, op1=Alu.add)
            with tc.tile_pool(name="psB", bufs=1, space="PSUM") as psB:
                ar_ps = psB.tile([8, 8], dt.float32, tag="ar")
                nc.tensor.matmul(ar_ps[:], tmp8[:], tmpk8[:], start=True,
                                 stop=True)
                ar = smallp.tile([8, 8], dt.float32, tag="arsb")
                nc.vector.tensor_copy(ar[:], ar_ps[:])
            srt = smallp.tile([8, 8], dt.float32, tag="srt")
            nc.vector.max(srt[:], ar[:])
            msel = smallp.tile([8, 8], dt.float32, tag="msel")
            nc.vector.tensor_scalar(msel[:], ar[:], srt[:, 3:4], None,
                                    op0=Alu.is_ge)
            if debug:
                nc.sync.dma_start(dbg["m"][:], msel[:])
            with tc.tile_pool(name="psB2", bufs=1, space="PSUM") as psB2:
                o1 = psB2.tile([8, 64], dt.float32, tag="o1")
                nc.tensor.matmul(o1[:], msel[:], ro_sb[:, 0:64],
                                 start=True, stop=True)
                o1m = smallp.tile([8, 64], dt.float32, tag="o1m")
                nc.vector.tensor_tensor(o1m[:], o1[:], ro_sb[:, 64:128],
                                        op=Alu.mult)
                mb_ps = psB2.tile([128, 64], dt.float32, tag="mbps")
                nc.tensor.matmul(mb_ps[:], ro_sb[:, 128:256], o1m[:],
                                 start=True, stop=True)
                mbc = smallp.tile([128, 64], dt.float32, tag="mbc")
                nc.vector.tensor_copy(mbc[:], mb_ps[:])
            # scid8[c, (w*8+w')*128 + c'] = 0.25*mask (ident_sb = 0.25*I)
            for w in range(8):
                for w2 in range(8):
                    i = w * 8 + w2
                    if w % 2 == 0:
                        nc.vector.tensor_scalar(
                            scid8[:, i * 128:(i + 1) * 128], ident_sb[:],
                            mbc[:, i:i + 1], None, op0=Alu.mult)
                    else:
                        nc.scalar.activation(
                            scid8[:, i * 128:(i + 1) * 128], ident_sb[:],
                            Act.Identity, scale=mbc[:, i:i + 1])
            tc.strict_bb_all_engine_barrier()
            if debug:
                nc.sync.dma_start(dbg["scid_e"][:], scid8[:])

            # ============ STAGE C: pipelined LIF + gather + attn + out ======
            attn_half = {}
            uq_prev = [None]
            uk_prev = [None]
            uat_prev = [None]
            uz_prev = [None]
            p16_box = [None]

            from contextlib import ExitStack
            with ExitStack() as cstack:
                psKV = cstack.enter_context(
                    tc.tile_pool(name="psKV", bufs=2, space="PSUM"))
                psAt = cstack.enter_context(
                    tc.tile_pool(name="psAt", bufs=1, space="PSUM"))
                psEx = cstack.enter_context(
                    tc.tile_pool(name="psEx", bufs=1, space="PSUM"))
                psP = cstack.enter_context(
                    tc.tile_pool(name="psP", bufs=2, space="PSUM"))
                athfp = cstack.enter_context(tc.tile_pool(name="athf", bufs=1))
                atup = cstack.enter_context(tc.tile_pool(name="atuw", bufs=3))
                atwp = atup
                atsp = cstack.enter_context(tc.tile_pool(name="ats", bufs=2))
                qkp = cstack.enter_context(tc.tile_pool(name="qkp", bufs=2))
                yzp = cstack.enter_context(tc.tile_pool(name="zscr", bufs=3))
                zup = yzp
                zwp = yzp
                ztp = cstack.enter_context(tc.tile_pool(name="ztp", bufs=2))
                qlifp = cstack.enter_context(tc.tile_pool(name="lif", bufs=6))
                klifp = qlifp
                def lif_qk(t):
                    # scaled fp16 LIF: U_t = 2^t u_t = W_{t-1} + Y_t,
                    # Y_t = 2^t*(a_h*lin + b_h); spike U>=2^t; W = U*[U<2^t]
                    for (lin, spk, base, upool, upr) in (
                            (qlin, qs8, 0, qlifp, uq_prev),
                            (klin, ks8, 8, klifp, uk_prev)):
                        ysl = wts(lin)[:, :, t]
                        yt = upool.tile([128, 2048], dt.float16, tag="qu")
                        nc.scalar.activation(
                            v3(yt), ysl, Act.Identity,
                            bias=aff2[:, base + 2 * t + 1:base + 2 * t + 2],
                            scale=aff2[:, base + 2 * t:base + 2 * t + 1])
                        if t == 0:
                            uap = yt[:]
                        else:
                            ut = upool.tile([128, 2048], dt.float16, tag="qu")
                            nc.vector.tensor_tensor(ut[:], upr[0], yt[:],
                                                    op=Alu.add)
                            uap = ut[:]
                        nc.vector.tensor_scalar(
                            wts(spk)[:, :, t],
                            uap.rearrange("p (w s) -> p w s", w=8, s=256),
                            float(2.0 ** t), None, op0=Alu.is_ge)
                        if t < T - 1:
                            mk = upool.tile([128, 2048], dt.float16, tag="qu")
                            nc.vector.tensor_scalar(
                                mk[:], uap, float(2.0 ** t), None,
                                op0=Alu.is_lt)
                            wt_ = upool.tile([128, 2048], dt.float16,
                                             tag="qu")
                            nc.vector.tensor_tensor(wt_[:], uap, mk[:],
                                                    op=Alu.mult)
                            upr[0] = wt_[:]

                def gather_half(half):
                    at_sb = athfp.tile([8, 4096], dt.float16, tag="athf")
                    attn_half[half] = at_sb
                    ks8v = ks8[:].rearrange("p (w hs) -> p w hs", w=8, hs=1024)
                    vs8v = vs8[:].rearrange("p (w hs) -> p w hs", w=8, hs=1024)
                    sc8v = scid8[:].rearrange("p (b c) -> p b c", b=64, c=128)
                    hs = slice(half * 512, half * 512 + 512)
                    for w in range(NUM_WINS):
                        kag = psKV.tile([128, 512], dt.float32, tag="kag")
                        vag = psKV.tile([128, 512], dt.float32, tag="vag")
                        for ps, src in ((kag, ks8v), (vag, vs8v)):
                            for pr in range(4):
                                lhs = sc8v[:, w * 8 + 2 * pr:
                                           w * 8 + 2 * pr + 2]
                                rhs = src[:, 2 * pr:2 * pr + 2, hs]
                                nc.tensor.matmul(ps[:], lhs, rhs,
                                                 start=(pr == 0),
                                                 stop=(pr == 3),
                                                 perf_mode=DR)
                        nc.scalar.activation(
                            vag8[:, w * 1024 + half * 512:
                                 w * 1024 + half * 512 + 512],
                            vag[:], Act.Copy)
                        qk = qkp.tile([128, 512], dt.bfloat16, tag="qk")
                        nc.vector.tensor_tensor(
                            qk[:], qs8[:, w * 1024 + half * 512:
                                        w * 1024 + half * 512 + 512],
                            kag[:], op=Alu.mult)
                        if debug and half == 0 and w == 0:
                            nc.sync.dma_start(dbg["qk"][:], qk[:])
                        at = psAt.tile([8, 512], dt.float32, tag="at")
                        for ts_ in range(2):
                            t = 2 * half + ts_
                            nc.tensor.matmul(
                                at[:, ts_ * 256:(ts_ + 1) * 256],
                                amats_sb[:, t * 8:t * 8 + 8],
                                qk[:, ts_ * 256:(ts_ + 1) * 256],
                                start=True, stop=True)
                        nc.scalar.activation(
                            at_sb[:, w * 512:(w + 1) * 512], at[:], Act.Copy)

                def attn_lif(t):
                    half, ts_ = t // 2, t % 2
                    ysl = attn_half[half][:].rearrange(
                        "p (w u s) -> p w u s", w=8, u=2, s=256)[:, :, ts_]
                    if t == 0:
                        uap = ysl
                    else:
                        ut = atup.tile([8, 2048], dt.float16, tag="atu")
                        nc.vector.tensor_tensor(
                            ut[:].rearrange("p (w s) -> p w s", w=8, s=256),
                            uat_prev[0], ysl, op=Alu.add)
                        uap = ut[:].rearrange("p (w s) -> p w s", w=8, s=256)
                    at_s = atsp.tile([8, 2048], dt.float8e4, tag="ats")
                    nc.vector.tensor_scalar(
                        at_s[:].rearrange("p (w s) -> p w s", w=8, s=256),
                        uap, P2[t], None, op0=Alu.is_ge)
                    if t < T - 1:
                        wt_ = atwp.tile([8, 2048], dt.float16, tag="atu")
                        nc.vector.scalar_tensor_tensor(
                            wt_[:].rearrange("p (w s) -> p w s", w=8, s=256),
                            uap, P2[t], uap, op0=Alu.is_lt, op1=Alu.mult)
                        uat_prev[0] = wt_[:].rearrange(
                            "p (w s) -> p w s", w=8, s=256)
                    return at_s

                def z_stage(t, at_s):
                    p16 = p16_box[0]
                    yz = yzp.tile([128, 2048], dt.bfloat16, tag="yz")
                    at_v = at_s[:].rearrange("p (w s) -> p w s", w=8, s=256)
                    for wp in range(4):
                        ex = psEx.tile([128, 512], dt.float32, tag="ex")
                        nc.tensor.matmul(ex[:], emat_sb[:],
                                         at_v[:, 2 * wp:2 * wp + 2],
                                         start=True, stop=True)
                        vsl = wts(vag8)[:, 2 * wp:2 * wp + 2, t]
                        ydst = yz[:, wp * 512:(wp + 1) * 512].rearrange(
                            "p (w s) -> p w s", w=2, s=256)
                        nc.vector.scalar_tensor_tensor(
                            ydst, ex[:].rearrange("p (w s) -> p w s",
                                                  w=2, s=256),
                            0.5 * P2[t], vsl, op0=Alu.mult, op1=Alu.mult)
                    if t == 0:
                        uap = yz[:]
                    else:
                        ut = zup.tile([128, 2048], dt.bfloat16, tag="yz")
                        nc.vector.tensor_tensor(ut[:], uz_prev[0], yz[:],
                                                op=Alu.add)
                        uap = ut[:]
                    zt = ztp.tile([128, 2048], dt.bfloat16, tag="zt")
                    nc.vector.tensor_scalar(zt[:], uap, P2[t], None,
                                            op0=Alu.is_ge)
                    if t < T - 1:
                        mkz = zwp.tile([128, 2048], dt.bfloat16, tag="yz")
                        nc.vector.tensor_scalar(mkz[:], uap, P2[t], None,
                                                op0=Alu.is_lt)
                        wt_ = zwp.tile([128, 2048], dt.bfloat16, tag="yz")
                        nc.vector.tensor_tensor(wt_[:], uap, mkz[:],
                                                op=Alu.mult)
                        uz_prev[0] = wt_[:]
                    if debug:
                        nc.sync.dma_start(dbg["z"][t], zt[:])
                    ztv = zt[:].rearrange("p (w dh dw) -> p w dh dw",
                                          w=8, dh=16, dw=16)
                    for g in range(2):
                        for hh in range(2):
                            w0 = 4 * g + 2 * hh
                            rhs = ztv[:, w0:w0 + 2].rearrange(
                                "p ww dh dw -> p dh ww dw")
                            pp = psP.tile([128, 512], dt.float32, tag="pp")
                            nc.tensor.matmul(pp[:], pwT_sb[:, 0:128], rhs,
                                             start=True, stop=True)
                            slot = (t * 2 + g) * 2 + hh
                            dst = p16[:, t * 2048 + g * 1024 + hh * 512:
                                      t * 2048 + g * 1024 + hh * 512 + 512]
                            nc.scalar.activation(
                                dst, pp[:], Act.Copy,
                                accum_out=psums[:, slot:slot + 1])
                            pscr = scr512.tile([128, 512], dt.float32,
                                               tag="sq512")
                            nc.scalar.activation(
                                pscr[:], pp[:], Act.Square,
                                accum_out=psq[:, slot:slot + 1])

                lif_qk(0)
                lif_qk(1)
                gather_half(0)
                if debug:
                    nc.sync.dma_start(dbg["vag_e"][:],
                                      wts(vag8)[:, :, 0:2])
                    nc.sync.dma_start(dbg["attn_e"][:], attn_half[0][:])
                lif_qk(2)
                lif_qk(3)
                if debug:
                    nc.sync.dma_start(dbg["qlin"][:], qlin[:])
                    nc.sync.dma_start(dbg["klin"][:], klin[:])
                p16_box[0] = p16g
                z_stage(0, attn_lif(0))
                gather_half(1)
                z_stage(1, attn_lif(1))
                for t in (2, 3):
                    z_stage(t, attn_lif(t))

                if debug:
                    nc.sync.dma_start(dbg["vag"][:], vag8[:])
                    nc.sync.dma_start(dbg["scid"][:], scid8[:])
                    nc.sync.dma_start(dbg["qs"][:], qs8[:])
                    nc.sync.dma_start(dbg["ks"][:], ks8[:])
                    nc.sync.dma_start(dbg["vs"][:], vs8[:])
                    nc.sync.dma_start(dbg["attn"][:, 0:4096], attn_half[0][:])
                    nc.sync.dma_start(dbg["attn"][:, 4096:8192],
                                      attn_half[1][:])
                    nc.sync.dma_start(dbg["p"][:], p16_box[0][:])

                # ============ STAGE D: AR2 + final affine + out DMA ========
                pay2 = smallp.tile([128, 8], dt.float32, tag="pay2")
                nc.vector.tensor_reduce(pay2[:, 0:1], psums[:],
                                        axis=mybir.AxisListType.X, op=Alu.add)
                nc.vector.tensor_reduce(pay2[:, 1:2], psq[:],
                                        axis=mybir.AxisListType.X, op=Alu.add)
                nc.gpsimd.memset(pay2[:, 2:8], 0.0)
                cc2_in = dramp.tile([128, 8], dt.float32, tag="cc2_in")
                cc2_out = dramp.tile([128, 8], dt.float32, tag="cc2_out")
                nc.sync.dma_start(cc2_in[:], pay2[:])
                nc.gpsimd.collective_compute(
                    "AllReduce", Alu.add, replica_groups=RG,
                    ins=[cc2_in.opt()], outs=[cc2_out.opt()])
                S2 = smallp.tile([128, 8], dt.float32, tag="S2")
                nc.sync.dma_start(S2[:], cc2_out[:])

                meanp = smallp.tile([128, 1], dt.float32, tag="meanp")
                nc.vector.tensor_scalar(meanp[:], S2[:, 0:1], 1.0 / NTOT,
                                        None, op0=Alu.mult)
                varp = smallp.tile([128, 1], dt.float32, tag="varp")
                nc.vector.tensor_scalar(varp[:], S2[:, 1:2], 1.0 / NTOT,
                                        None, op0=Alu.mult)
                msqp = smallp.tile([128, 1], dt.float32, tag="msqp")
                nc.vector.tensor_tensor(msqp[:], meanp[:], meanp[:],
                                        op=Alu.mult)
                nc.vector.tensor_tensor(varp[:], varp[:], msqp[:],
                                        op=Alu.subtract)
                nc.vector.tensor_scalar(varp[:], varp[:], EPS, None,
                                        op0=Alu.add)
                nc.scalar.sqrt(varp[:], varp[:])
                rstdp = smallp.tile([128, 1], dt.float32, tag="rstdp")
                nc.vector.reciprocal(rstdp[:], varp[:])
                ap_ = smallp.tile([128, 1], dt.float32, tag="ap_")
                nc.vector.tensor_tensor(ap_[:], gb_sb[:, 6:7], rstdp[:],
                                        op=Alu.mult)
                bp_ = smallp.tile([128, 1], dt.float32, tag="bp_")
                nc.vector.tensor_tensor(bp_[:], ap_[:], meanp[:], op=Alu.mult)
                nc.vector.tensor_tensor(bp_[:], gb_sb[:, 7:8], bp_[:],
                                        op=Alu.subtract)

                with tc.tile_pool(name="outp", bufs=2) as outp:
                    for t in range(T):
                        for g in range(2):
                            oft = outp.tile([128, 1024], dt.bfloat16,
                                            tag="of")
                            of = oft[:]
                            src = p16_box[0][:, t * 2048 + g * 1024:
                                             t * 2048 + (g + 1) * 1024]
                            if (t * 2 + g) % 2 == 0:
                                nc.vector.tensor_scalar(
                                    of, src, ap_[:], bp_[:],
                                    op0=Alu.mult, op1=Alu.add)
                            else:
                                nc.scalar.activation(
                                    of, src, Act.Identity,
                                    bias=bp_[:], scale=ap_[:])
                            eng = (nc.sync, nc.scalar)[(t * 2 + g) % 2]
                            eng.dma_start(out_d[t, g], of)
            latestack.close()

    nc.compile()
    return nc


def _host_inputs(x, qw, q_gamma, q_beta, kw, k_gamma, k_beta,
                 v_gamma, v_beta, pw, p_gamma, p_beta):
    """Build the 8 per-core input dicts."""
    f32 = np.float32
    bf16 = ml_dtypes.bfloat16
    f8 = ml_dtypes.float8_e4m3
    x = np.ascontiguousarray(x, f32)
    qw = np.asarray(qw, f32)
    kw = np.asarray(kw, f32)
    pw = np.asarray(pw, f32)

    kd = (qw[:, :, 0] + qw[:, :, 2]).sum((-1, -2))  # [O, I]
    qw_eff = qw.copy()
    qw_eff[:, :, 1, 1, 1] -= THETA * kd
    w27 = qw_eff.reshape(128, 128, 27).transpose(1, 2, 0).reshape(128, 27 * 128)
    w27 = np.ascontiguousarray(w27, f32)
    kwT = np.ascontiguousarray(kw.T, f32)
    pwT = pw.T  # [i, o]
    pw_hi = pwT.astype(bf16)
    pw_lo = (pwT - pw_hi.astype(f32)).astype(bf16)
    pwT2 = np.stack([pw_hi, pw_lo])

    gb = np.stack([q_gamma, q_beta, k_gamma, k_beta, v_gamma, v_beta,
                   p_gamma, p_beta], axis=1).astype(f32)
    ident = (0.25 * np.eye(128)).astype(bf16)
    amats = np.zeros((128, 32), bf16)
    for c in range(128):
        for t in range(T):
            amats[c, t * 8 + c // 16] = 0.5 * (2.0 ** t)
    emat8 = np.zeros((8, 128), f8)
    for c in range(128):
        emat8[c // 16, c] = 1.0

    # x windowed: [t, b, c, wt, dt, wh, dh, ww, dw]
    xw = x.reshape(T, B, C, 2, 4, 2, LH, 2, LW)

    in_maps = []
    for core in range(8):
        b, j = core // 4, core % 4
        xconv = np.zeros((T, 2, 3, 128, 34, 34), f32)
        for g in range(2):
            for dd in range(3):
                d = j + 4 * g + dd - 1
                if 0 <= d < D:
                    xconv[:, g, dd, :, 1:33, 1:33] = x[:, b, :, d]
        # xv[c, w, t, s]: w = wt*4+wh*2+ww, s = dh*16+dw, dt=j
        xvw = xw[:, b, :, :, j]  # [t, c, wt, wh, dh, ww, dw]
        xvw = xvw.transpose(1, 2, 3, 5, 0, 4, 6)  # [c, wt, wh, ww, t, dh, dw]
        xv = xvw.reshape(C, NUM_WINS, T, 256)
        bm16 = np.zeros((128, 16), f32)
        bm16[:, b * 8:(b + 1) * 8] = 1.0
        romats = np.zeros((8, 256), f32)
        for k in range(8):
            for w in range(8):
                for w2 in range(8):
                    if k == w:
                        romats[k, w * 8 + w2] = 1.0
                    if k == w2:
                        romats[k, 64 + w * 8 + w2] = 1.0
        romats[:, 128:256] = 1.0
        in_maps.append({
            "xconv": np.ascontiguousarray(xconv.reshape(T, 2, 3, 128, 34 * 34)),
            "xv": np.ascontiguousarray(xv),
            "w27": w27, "kwT": kwT, "pwT2": pwT2, "gb": gb,
            "identw": ident, "amats": amats, "emat8": emat8, "bmask16": bm16,
            "romats": romats,
        })
    return in_maps


def kernel(**inputs):
    from concourse.bass_utils import run_bass_kernel_spmd

    key = ("dbg" if DEBUG else "plain")
    if key not in _COMPILED:
        _COMPILED[key] = _build(DEBUG)
    nc = _COMPILED[key]

    in_maps = _host_inputs(**inputs)
    res = run_bass_kernel_spmd(nc, in_maps, core_ids=list(range(8)))
    kernel.last_results = res

    full = np.empty((T, B, C, D, H, W), np.float32)
    for core in range(8):
        b, j = core // 4, core % 4
        oc = np.asarray(res.results[core]["out"], dtype=np.float32)
        for g in range(2):
            full[:, b, :, j + 4 * g] = oc[:, g].reshape(T, C, H, W)
    return full


# revision 43
# speedup vs baseline: 1.0079x; 1.0079x over previous
"""Trainium2 Bass kernel for nn_BiSDA (spiking bi-directional sparse attention).

v2 strategy (8 NeuronCores, single SPMD launch), core c = b*4 + j:
  - Layout: all big per-core tensors are [128, 8192] with free index
    w*1024 + t*256 + s  (window-major, s = dh*16+dw within window, dt=j).
  - v-chain (lif(bn(x))) depends only on x: its BN stats AllReduce runs at
    kernel start (also warming the collective path) and the whole v-LIF
    executes on DVE underneath the conv (PE-bound) phase.
  - Conv psum evacuation runs on ACT (Copy + accum_out sum stats); squares
    run as tensor_tensor_reduce on DVE. No DVE copies.
  - Gather-mean over routed windows: fp8e4 DoubleRow matmuls (two stacked
    128-contractions per instruction, 0.5 cyc/row) with mask-scaled
    0.25-identity lhs; spikes are written directly as fp8 (exact).
  - Attention dot: amat_t lhs columns carry 0.5*2^t so the attention LIF
    runs in scaled form (U_t = 2^t u_t), bit-exact in fp16; the z LIF is
    likewise scaled and exact in bf16.
  - q/k LIF stay fp32: q on DVE, k u/W-updates on Pool (k spike on DVE).
  - Final projection: exact bf16 hi/lo split of pw against binary spikes;
    stats via ACT accum; second AllReduce; affine + streamed output DMA.
"""

import os
import sys

import numpy as np

sys.path.insert(0, "/opt/trn_rl_repo")

import ml_dtypes  # noqa: E402

T, B, C = 4, 2, 128
D, H, W = 8, 32, 32
NUM_WINS = 8
LH, LW = 16, 16
NUM_HEADS, HEAD_DIM = 8, 16
THETA = 0.7
EPS = 1e-5
NTOT = float(T * B * D * H * W)
REGION_N = float(T * 4 * LH * LW)

DEBUG = bool(int(os.environ.get("BISDA_DEBUG", "0")))

_COMPILED = {}


def _build(debug):
    import concourse.bacc as bacc
    import concourse.mybir as mybir
    from concourse import tile

    dt = mybir.dt
    Alu = mybir.AluOpType
    Act = mybir.ActivationFunctionType
    DR = mybir.MatmulPerfMode.DoubleRow

    nc = bacc.Bacc("TRN2", target_bir_lowering=False, debug=False,
                   enable_asserts=False, num_devices=8)

    # ---------------- DRAM I/O ----------------
    xconv = nc.dram_tensor("xconv", [T, 2, 3, 128, 34 * 34], dt.float32r,
                           kind="ExternalInput")
    xv = nc.dram_tensor("xv", [128, NUM_WINS, T, 256], dt.float32,
                        kind="ExternalInput")
    w27 = nc.dram_tensor("w27", [128, 27 * 128], dt.float32r, kind="ExternalInput")
    kwT = nc.dram_tensor("kwT", [128, 128], dt.float32r, kind="ExternalInput")
    pwT2 = nc.dram_tensor("pwT2", [2, 128, 128], dt.bfloat16, kind="ExternalInput")
    # gb columns: q_gamma,q_beta,k_gamma,k_beta,v_gamma,v_beta,p_gamma,p_beta
    gb = nc.dram_tensor("gb", [128, 8], dt.float32, kind="ExternalInput")
    identw = nc.dram_tensor("identw", [128, 128], dt.bfloat16,
                            kind="ExternalInput")  # 0.25 * I
    amats = nc.dram_tensor("amats", [128, 32], dt.bfloat16,
                           kind="ExternalInput")  # col t*8+h: 0.5*2^t one-hot
    emat8 = nc.dram_tensor("emat8", [8, 128], dt.float8e4,
                           kind="ExternalInput")  # 1.0 one-hot expand
    bmask16 = nc.dram_tensor("bmask16", [128, 16], dt.float32,
                             kind="ExternalInput")
    # routing broadcast consts: cols 0:64 wsel[k,(w,w')]= (k==w);
    # 64:128 mask1[k,(w,w')] = (k==w'); 128:256 ones
    romats = nc.dram_tensor("romats", [8, 256], dt.float32,
                            kind="ExternalInput")

    out_d = nc.dram_tensor("out", [T, 2, 128, 1024], dt.bfloat16,
                           kind="ExternalOutput")
    dbg = {}
    if debug:
        dbg["qlin"] = nc.dram_tensor("dbg_qlin", [128, 8192], dt.float32,
                                     kind="ExternalOutput")
        dbg["klin"] = nc.dram_tensor("dbg_klin", [128, 8192], dt.float32,
                                     kind="ExternalOutput")
        dbg["stats"] = nc.dram_tensor("dbg_stats", [128, 40], dt.float32,
                                      kind="ExternalOutput")
        dbg["m"] = nc.dram_tensor("dbg_m", [8, 8], dt.float32,
                                  kind="ExternalOutput")
        dbg["qs"] = nc.dram_tensor("dbg_qs", [128, 8192], dt.float8e4,
                                   kind="ExternalOutput")
        dbg["ks"] = nc.dram_tensor("dbg_ks", [128, 8192], dt.float8e4,
                                   kind="ExternalOutput")
        dbg["vs"] = nc.dram_tensor("dbg_vs", [128, 8192], dt.float8e4,
                                   kind="ExternalOutput")
        dbg["attn"] = nc.dram_tensor("dbg_attn", [8, 8192], dt.float16,
                                     kind="ExternalOutput")
        dbg["z"] = nc.dram_tensor("dbg_z", [4, 128, 2048], dt.bfloat16,
                                  kind="ExternalOutput")
        dbg["p"] = nc.dram_tensor("dbg_p", [128, 8192], dt.bfloat16,
                                  kind="ExternalOutput")
        dbg["vag"] = nc.dram_tensor("dbg_vag", [128, 8192], dt.float8e4,
                                    kind="ExternalOutput")
        dbg["qk"] = nc.dram_tensor("dbg_qk", [128, 512], dt.bfloat16,
                                   kind="ExternalOutput")
        dbg["scid"] = nc.dram_tensor("dbg_scid", [128, 8192], dt.float8e4,
                                     kind="ExternalOutput")
        dbg["scid_e"] = nc.dram_tensor("dbg_scid_e", [128, 8192], dt.float8e4,
                                       kind="ExternalOutput")
        dbg["vag_e"] = nc.dram_tensor("dbg_vag_e", [128, 8, 2, 256],
                                      dt.float8e4, kind="ExternalOutput")
        dbg["attn_e"] = nc.dram_tensor("dbg_attn_e", [8, 4096], dt.float16,
                                       kind="ExternalOutput")

    RG = [[0, 1, 2, 3, 4, 5, 6, 7]]
    P2 = [1.0, 2.0, 4.0, 8.0]  # 2^t

    with tile.TileContext(nc) as tc:
        with (
            tc.tile_pool(name="const", bufs=1) as constp,
            tc.tile_pool(name="dram", bufs=1, space="DRAM") as dramp,
            tc.tile_pool(name="big", bufs=1) as bigp,
            tc.tile_pool(name="small", bufs=1) as smallp,
            tc.tile_pool(name="scr512", bufs=2) as scr512,
        ):
            # ---- constants ----
            kwT_sb = constp.tile([128, 128], dt.float32r, tag="kwT")
            nc.sync.dma_start(kwT_sb[:], kwT[:])
            pwT_sb = constp.tile([128, 256], dt.bfloat16, tag="pwT")
            nc.sync.dma_start(pwT_sb[:, 0:128], pwT2[0])
            nc.sync.dma_start(pwT_sb[:, 128:256], pwT2[1])
            gb_sb = constp.tile([128, 8], dt.float32, tag="gb")
            nc.sync.dma_start(gb_sb[:], gb[:])
            ident_sb = constp.tile([128, 128], dt.bfloat16, tag="ident")
            nc.sync.dma_start(ident_sb[:], identw[:])
            amats_sb = constp.tile([128, 32], dt.bfloat16, tag="amats")
            nc.sync.dma_start(amats_sb[:], amats[:])
            emat_sb = constp.tile([8, 128], dt.float8e4, tag="emat")
            nc.sync.dma_start(emat_sb[:], emat8[:])
            bm16_sb = constp.tile([128, 16], dt.float32, tag="bm16")
            nc.sync.dma_start(bm16_sb[:], bmask16[:])
            ro_sb = constp.tile([8, 256], dt.float32, tag="romats")
            nc.sync.dma_start(ro_sb[:], romats[:])

            # persistent big tensors
            qlin = bigp.tile([128, 8192], dt.float32, tag="qlin")
            klin = bigp.tile([128, 8192], dt.float32, tag="klin")
            vs8 = bigp.tile([128, 8192], dt.float8e4, tag="vs8")

            def wts(big):
                return big[:].rearrange("p (w t s) -> p w t s",
                                        w=8, t=4, s=256)

            def v3(tl):
                return tl[:].rearrange("p (w s) -> p w s", w=8, s=256)

            sums = smallp.tile([128, 72], dt.float32, tag="sums")
            # qsum 0:16, qsq 16:32, ksum 32:48, ksq 48:64, xsum 64:66,
            # xsq 66:68  (q/k slots: (t*2+g)*2+hh)
            qregs = smallp.tile([128, 32], dt.float32, tag="qregs")
            kregs = smallp.tile([128, 32], dt.float32, tag="kregs")
            psums = smallp.tile([128, 16], dt.float32, tag="psums")
            psq = smallp.tile([128, 16], dt.float32, tag="psq")
            aff = smallp.tile([128, 16], dt.float32, tag="aff")
            # aff cols: 0 aq_h,1 bq_h,2 ak_h,3 bk_h,4 av_h,5 bv_h,
            #           6 aq_f,7 bq_f,8 ak_f,9 bk_f

            def bn_affine(scol, gcol, St_tile, dst_half, dst_full, name):
                mean = smallp.tile([128, 1], dt.float32, tag=f"mean{name}")
                nc.vector.tensor_scalar(mean[:], St_tile[:, scol:scol + 1],
                                        1.0 / NTOT, None, op0=Alu.mult)
                var = smallp.tile([128, 1], dt.float32, tag=f"var{name}")
                nc.vector.tensor_scalar(var[:], St_tile[:, scol + 1:scol + 2],
                                        1.0 / NTOT, None, op0=Alu.mult)
                msq = smallp.tile([128, 1], dt.float32, tag=f"msq{name}")
                nc.vector.tensor_tensor(msq[:], mean[:], mean[:], op=Alu.mult)
                nc.vector.tensor_tensor(var[:], var[:], msq[:], op=Alu.subtract)
                nc.vector.tensor_scalar(var[:], var[:], EPS, None, op0=Alu.add)
                nc.scalar.sqrt(var[:], var[:])
                rstd = smallp.tile([128, 1], dt.float32, tag=f"rstd{name}")
                nc.vector.reciprocal(rstd[:], var[:])
                afull = smallp.tile([128, 1], dt.float32, tag=f"af{name}")
                nc.vector.tensor_tensor(afull[:], gb_sb[:, gcol:gcol + 1],
                                        rstd[:], op=Alu.mult)
                bfull = smallp.tile([128, 1], dt.float32, tag=f"bf{name}")
                nc.vector.tensor_tensor(bfull[:], afull[:], mean[:], op=Alu.mult)
                nc.vector.tensor_tensor(bfull[:], gb_sb[:, gcol + 1:gcol + 2],
                                        bfull[:], op=Alu.subtract)
                nc.vector.tensor_scalar(aff[:, dst_half:dst_half + 1], afull[:],
                                        0.5, None, op0=Alu.mult)
                nc.vector.tensor_scalar(aff[:, dst_half + 1:dst_half + 2],
                                        bfull[:], 0.5, None, op0=Alu.mult)
                if dst_full is not None:
                    nc.vector.tensor_copy(aff[:, dst_full:dst_full + 1], afull[:])
                    nc.vector.tensor_copy(aff[:, dst_full + 1:dst_full + 2],
                                          bfull[:])

            # ============ STAGE 0 + A: xv, x-stats, AR_v, conv ============
            with (
                tc.tile_pool(name="xvp", bufs=1) as xvp,
                tc.tile_pool(name="vlif", bufs=3) as vlifp,
                tc.tile_pool(name="xin", bufs=6) as xinp,
                tc.tile_pool(name="psA", bufs=6, space="PSUM") as psA,
                tc.tile_pool(name="psK", bufs=2, space="PSUM") as psK,
                tc.tile_pool(name="w27p", bufs=1) as w27p,
            ):
                w27_sb = w27p.tile([128, 27 * 128], dt.float32r, tag="w27")
                nc.sync.dma_start(w27_sb[:, 0:9 * 128], w27[:, 0:9 * 128])
                xpads0 = {}
                for dd in range(3):
                    for g in range(2):
                        xp = xinp.tile([128, 34 * 34], dt.float32r, tag="xpad")
                        nc.sync.dma_start(xp[:], xconv[0, g, dd])
                        xpads0[(g, dd)] = xp
                nc.sync.dma_start(w27_sb[:, 9 * 128:27 * 128],
                                  w27[:, 9 * 128:27 * 128])
                xv_sb = xvp.tile([128, 8192], dt.float32, tag="xv")

                # ---- conv ----
                for t in range(T):
                    if t == 3:
                        nc.sync.dma_start(
                            xv_sb[:],
                            xv[:].rearrange("p w t s -> p (w t s)"))
                        for q4 in range(4):
                            xsl = xv_sb[:, q4 * 2048:(q4 + 1) * 2048]
                            xsc = vlifp.tile([128, 2048], dt.float32,
                                             tag="vu")
                            nc.scalar.activation(
                                xsc[:], xsl, Act.Copy,
                                accum_out=sums[:, 64 + q4:65 + q4])
                            xsc2 = vlifp.tile([128, 2048], dt.float32,
                                              tag="vu")
                            nc.scalar.activation(
                                xsc2[:], xsl, Act.Square,
                                accum_out=sums[:, 68 + q4:69 + q4])
                        pay_v = smallp.tile([128, 8], dt.float32, tag="pay_v")
                        nc.gpsimd.memset(pay_v[:], 0.0)
                        nc.vector.tensor_reduce(
                            pay_v[:, 0:1], sums[:, 64:68],
                            axis=mybir.AxisListType.X, op=Alu.add)
                        nc.vector.tensor_reduce(
                            pay_v[:, 1:2], sums[:, 68:72],
                            axis=mybir.AxisListType.X, op=Alu.add)
                        ccv_in = dramp.tile([128, 8], dt.float32,
                                            tag="ccv_in")
                        ccv_out = dramp.tile([128, 8], dt.float32,
                                             tag="ccv_out")
                        nc.sync.dma_start(ccv_in[:], pay_v[:])
                        nc.gpsimd.collective_compute(
                            "AllReduce", Alu.add, replica_groups=RG,
                            ins=[ccv_in.opt()], outs=[ccv_out.opt()])
                        Sv = smallp.tile([128, 8], dt.float32, tag="Sv")
                        nc.sync.dma_start(Sv[:], ccv_out[:])
                        bn_affine(0, 4, Sv, 4, None, "v")
                    if t == 0:
                        xpads = xpads0
                    else:
                        xpads = {}
                        for dd in range(3):
                            for g in range(2):
                                xp = xinp.tile([128, 34 * 34], dt.float32r,
                                               tag="xpad")
                                nc.sync.dma_start(xp[:], xconv[t, g, dd])
                                xpads[(g, dd)] = xp
                    pss = {}
                    for g in range(2):
                        for hh in range(2):
                            psx = psA.tile([128, 512], dt.float32, tag="psA")
                            pss[(g, hh)] = psx
                    for tap in range(27):
                        dd, rem = divmod(tap, 9)
                        dhh, dww = divmod(rem, 3)
                        lhs = w27_sb[:, tap * 128:(tap + 1) * 128]
                        for g in range(2):
                            for hh in range(2):
                                rhs = xpads[(g, dd)][:].rearrange(
                                    "p (r c) -> p r c", r=34, c=34
                                )[:, 16 * hh + dhh:16 * hh + dhh + 16,
                                  dww:dww + 32]
                                nc.tensor.matmul(pss[(g, hh)][:], lhs, rhs,
                                                 start=(tap == 0),
                                                 stop=(tap == 26))
                    # k_lin from center planes, rhs (ww, dh, dw)
                    for g in range(2):
                        for hh in range(2):
                            slot = (t * 2 + g) * 2 + hh
                            psk = psK.tile([128, 512], dt.float32, tag="psK")
                            ctr = xpads[(g, 1)][:].rearrange(
                                "p (r c) -> p r c", r=34, c=34)
                            rhs = ctr[:, 16 * hh + 1:16 * hh + 17, 1:33]
                            rhs = rhs.rearrange(
                                "p dh (ww dw) -> p ww dh dw", ww=2, dw=16)
                            nc.tensor.matmul(psk[:], kwT_sb[:], rhs,
                                             start=True, stop=True)
                            w0 = 4 * g + 2 * hh
                            kdst = wts(klin)[:, w0:w0 + 2, t]
                            nc.scalar.activation(
                                kdst,
                                psk[:].rearrange("p (ww s) -> p ww s",
                                                 ww=2, s=256),
                                Act.Copy,
                                accum_out=sums[:, 32 + slot:33 + slot])
                            ksc = scr512.tile([128, 512], dt.float32,
                                              tag="sq512")
                            nc.scalar.activation(
                                ksc[:], psk[:], Act.Square,
                                accum_out=sums[:, 48 + slot:49 + slot])
                        rc = g * 16 + t * 4
                        nc.vector.tensor_reduce(
                            kregs[:, rc:rc + 4],
                            wts(klin)[:, 4 * g:4 * g + 4, t],
                            axis=mybir.AxisListType.X, op=Alu.add)
                    for g in range(2):
                        for hh in range(2):
                            slot = (t * 2 + g) * 2 + hh
                            w0 = 4 * g + 2 * hh
                            dstap = wts(qlin)[:, w0:w0 + 2, t].rearrange(
                                "p ww (dh dw) -> p dh ww dw", dh=16, dw=16)
                            nc.scalar.activation(
                                dstap, pss[(g, hh)][:].rearrange(
                                    "p (dh ww dw) -> p dh ww dw",
                                    ww=2, dh=16, dw=16),
                                Act.Copy,
                                accum_out=sums[:, slot:slot + 1])
                            qsc = scr512.tile([128, 512], dt.float32,
                                              tag="sq512")
                            nc.scalar.activation(
                                qsc[:], pss[(g, hh)][:], Act.Square,
                                accum_out=sums[:, 16 + slot:17 + slot])
                        rc = g * 16 + t * 4
                        nc.vector.tensor_reduce(
                            qregs[:, rc:rc + 4],
                            wts(qlin)[:, 4 * g:4 * g + 4, t],
                            axis=mybir.AxisListType.X, op=Alu.add)

                    if t == 3:
                        wv = None
                        for tv in range(T):
                            yv = vlifp.tile([128, 2048], dt.float32, tag="vu")
                            nc.scalar.activation(
                                v3(yv), wts(xv_sb)[:, :, tv],
                                Act.Identity, bias=aff[:, 5:6],
                                scale=aff[:, 4:5])
                            if tv == 0:
                                u = yv
                            else:
                                un = vlifp.tile([128, 2048], dt.float32,
                                                tag="vu")
                                nc.vector.scalar_tensor_tensor(
                                    un[:], wv[:], 0.5, yv[:],
                                    op0=Alu.mult, op1=Alu.add)
                                u = un
                            nc.vector.tensor_scalar(
                                wts(vs8)[:, :, tv], v3(u), 1.0, None,
                                op0=Alu.is_ge)
                            if tv < T - 1:
                                wn = vlifp.tile([128, 2048], dt.float32,
                                                tag="vu")
                                nc.vector.scalar_tensor_tensor(
                                    wn[:], u[:], 1.0, u[:],
                                    op0=Alu.is_lt, op1=Alu.mult)
                                wv = wn

            # ============ STAGE B: AR1 + affines + routing ============
            from contextlib import ExitStack
            latestack = ExitStack()
            latep = latestack.enter_context(tc.tile_pool(name="late", bufs=1))
            p16g = latep.tile([128, 8192], dt.bfloat16, tag="p16")
            qs8 = latep.tile([128, 8192], dt.float8e4, tag="qs8")
            ks8 = latep.tile([128, 8192], dt.float8e4, tag="ks8")
            scid8 = latep.tile([128, 8192], dt.float8e4, tag="scid8")
            vag8 = latep.tile([128, 8192], dt.float8e4, tag="vag8")

            payload = smallp.tile([128, 40], dt.float32, tag="payload")
            for col, (base, cnt) in enumerate(
                    [(0, 16), (16, 16), (32, 16), (48, 16)]):
                nc.vector.tensor_reduce(
                    payload[:, col:col + 1], sums[:, base:base + cnt],
                    axis=mybir.AxisListType.X, op=Alu.add)
            qreg8 = smallp.tile([128, 8], dt.float32, tag="qreg8")
            nc.vector.tensor_reduce(
                qreg8[:].rearrange("p (g w) -> p g w", g=2, w=4),
                qregs[:].rearrange("p (g t w) -> p g w t", g=2, t=4, w=4),
                axis=mybir.AxisListType.X, op=Alu.add)
            kreg8 = smallp.tile([128, 8], dt.float32, tag="kreg8")
            nc.vector.tensor_reduce(
                kreg8[:].rearrange("p (g w) -> p g w", g=2, w=4),
                kregs[:].rearrange("p (g t w) -> p g w t", g=2, t=4, w=4),
                axis=mybir.AxisListType.X, op=Alu.add)
            nc.vector.tensor_tensor(payload[:, 4:12], qreg8[:],
                                    bm16_sb[:, 0:8], op=Alu.mult)
            nc.vector.tensor_tensor(payload[:, 12:20], qreg8[:],
                                    bm16_sb[:, 8:16], op=Alu.mult)
            nc.vector.tensor_tensor(payload[:, 20:28], kreg8[:],
                                    bm16_sb[:, 0:8], op=Alu.mult)
            nc.vector.tensor_tensor(payload[:, 28:36], kreg8[:],
                                    bm16_sb[:, 8:16], op=Alu.mult)
            nc.gpsimd.memset(payload[:, 36:40], 0.0)

            cc1_in = dramp.tile([128, 40], dt.float32, tag="cc1_in")
            cc1_out = dramp.tile([128, 40], dt.float32, tag="cc1_out")
            nc.sync.dma_start(cc1_in[:], payload[:])
            nc.gpsimd.collective_compute(
                "AllReduce", Alu.add, replica_groups=RG,
                ins=[cc1_in.opt()], outs=[cc1_out.opt()])
            St = smallp.tile([128, 40], dt.float32, tag="St")
            nc.sync.dma_start(St[:], cc1_out[:])
            if debug:
                nc.sync.dma_start(dbg["stats"][:], St[:])

            bn_affine(0, 0, St, 0, 6, "q")
            bn_affine(2, 2, St, 2, 8, "k")
            aff2 = smallp.tile([128, 16], dt.float32, tag="aff2")
            for tt in range(4):
                for (cc, base) in ((0, 0), (2, 8)):
                    nc.vector.tensor_scalar(
                        aff2[:, base + 2 * tt:base + 2 * tt + 2],
                        aff[:, cc:cc + 2], float(2.0 ** tt), None,
                        op0=Alu.mult)

            # routing: region means -> BN -> select b -> a_r -> top4 mask
            qr16 = smallp.tile([128, 16], dt.float32, tag="qr16")
            nc.vector.tensor_scalar(qr16[:], St[:, 4:20], 1.0 / REGION_N, None,
                                    op0=Alu.mult)
            nc.scalar.activation(qr16[:], qr16[:], Act.Identity,
                                 bias=aff[:, 7:8], scale=aff[:, 6:7])
            kr16 = smallp.tile([128, 16], dt.float32, tag="kr16")
            nc.vector.tensor_scalar(kr16[:], St[:, 20:36], 1.0 / REGION_N, None,
                                    op0=Alu.mult)
            nc.scalar.activation(kr16[:], kr16[:], Act.Identity,
                                 bias=aff[:, 9:10], scale=aff[:, 8:9])
            tmp8 = smallp.tile([128, 8], dt.float32, tag="tmp8")
            nc.vector.scalar_tensor_tensor(
                tmp8[:], qr16[:, 0:8], bm16_sb[:, 0:1], qr16[:, 8:16],
                op0=Alu.mult, op1=Alu.bypass)
            nc.vector.scalar_tensor_tensor(
                tmp8[:], qr16[:, 8:16], bm16_sb[:, 8:9], tmp8[:],
                op0=Alu.mult, op1=Alu.add)
            tmpk8 = smallp.tile([128, 8], dt.float32, tag="tmpk8")
            nc.vector.scalar_tensor_tensor(
                tmpk8[:], kr16[:, 0:8], bm16_sb[:, 0:1], kr16[:, 8:16],
                op0=Alu.mult, op1=Alu.bypass)
            nc.vector.scalar_tensor_tensor(
                tmpk8[:], kr16[:, 8:16], bm16_sb[:, 8:9], tmpk8[:],
                op0=Alu.mult, op1=Alu.add)
            with tc.tile_pool(name="psB", bufs=1, space="PSUM") as psB:
                ar_ps = psB.tile([8, 8], dt.float32, tag="ar")
                nc.tensor.matmul(ar_ps[:], tmp8[:], tmpk8[:], start=True,
                                 stop=True)
                ar = smallp.tile([8, 8], dt.float32, tag="arsb")
                nc.vector.tensor_copy(ar[:], ar_ps[:])
            srt = smallp.tile([8, 8], dt.float32, tag="srt")
            nc.vector.max(srt[:], ar[:])
            msel = smallp.tile([8, 8], dt.float32, tag="msel")
            nc.vector.tensor_scalar(msel[:], ar[:], srt[:, 3:4], None,
                                    op0=Alu.is_ge)
            if debug:
                nc.sync.dma_start(dbg["m"][:], msel[:])
            with tc.tile_pool(name="psB2", bufs=1, space="PSUM") as psB2:
                o1 = psB2.tile([8, 64], dt.float32, tag="o1")
                nc.tensor.matmul(o1[:], msel[:], ro_sb[:, 0:64],
                                 start=True, stop=True)
                o1m = smallp.tile([8, 64], dt.float32, tag="o1m")
                nc.vector.tensor_tensor(o1m[:], o1[:], ro_sb[:, 64:128],
                                        op=Alu.mult)
                mb_ps = psB2.tile([128, 64], dt.float32, tag="mbps")
                nc.tensor.matmul(mb_ps[:], ro_sb[:, 128:256], o1m[:],
                                 start=True, stop=True)
                mbc = smallp.tile([128, 64], dt.float32, tag="mbc")
                nc.vector.tensor_copy(mbc[:], mb_ps[:])
            # scid8[c, (w*8+w')*128 + c'] = 0.25*mask (ident_sb = 0.25*I)
            for w in range(8):
                for w2 in range(8):
                    i = w * 8 + w2
                    if w % 2 == 0:
                        nc.vector.tensor_scalar(
                            scid8[:, i * 128:(i + 1) * 128], ident_sb[:],
                            mbc[:, i:i + 1], None, op0=Alu.mult)
                    else:
                        nc.scalar.activation(
                            scid8[:, i * 128:(i + 1) * 128], ident_sb[:],
                            Act.Identity, scale=mbc[:, i:i + 1])
            tc.strict_bb_all_engine_barrier()
            if debug:
                nc.sync.dma_start(dbg["scid_e"][:], scid8[:])

            # ============ STAGE C: pipelined LIF + gather + attn + out ======
            attn_half = {}
            uq_prev = [None]
            uk_prev = [None]
            uat_prev = [None]
            uz_prev = [None]
            p16_box = [None]

            from contextlib import ExitStack
            with ExitStack() as cstack:
                psKV = cstack.enter_context(
                    tc.tile_pool(name="psKV", bufs=2, space="PSUM"))
                psAt = cstack.enter_context(
                    tc.tile_pool(name="psAt", bufs=1, space="PSUM"))
                psEx = cstack.enter_context(
                    tc.tile_pool(name="psEx", bufs=1, space="PSUM"))
                psP = cstack.enter_context(
                    tc.tile_pool(name="psP", bufs=2, space="PSUM"))
                athfp = cstack.enter_context(tc.tile_pool(name="athf", bufs=1))
                atup = cstack.enter_context(tc.tile_pool(name="atuw", bufs=3))
                atwp = atup
                atsp = cstack.enter_context(tc.tile_pool(name="ats", bufs=2))
                qkp = cstack.enter_context(tc.tile_pool(name="qkp", bufs=2))
                yzp = cstack.enter_context(tc.tile_pool(name="zscr", bufs=3))
                zup = yzp
                zwp = yzp
                ztp = cstack.enter_context(tc.tile_pool(name="ztp", bufs=2))
                qlifp = cstack.enter_context(tc.tile_pool(name="lif", bufs=6))
                klifp = qlifp
                def lif_qk(t):
                    # scaled fp16 LIF: U_t = 2^t u_t = W_{t-1} + Y_t,
                    # Y_t = 2^t*(a_h*lin + b_h); spike U>=2^t; W = U*[U<2^t]
                    for (lin, spk, base, upool, upr) in (
                            (qlin, qs8, 0, qlifp, uq_prev),
                            (klin, ks8, 8, klifp, uk_prev)):
                        ysl = wts(lin)[:, :, t]
                        yt = upool.tile([128, 2048], dt.float16, tag="qu")
                        nc.scalar.activation(
                            v3(yt), ysl, Act.Identity,
                            bias=aff2[:, base + 2 * t + 1:base + 2 * t + 2],
                            scale=aff2[:, base + 2 * t:base + 2 * t + 1])
                        if t == 0:
                            uap = yt[:]
                        else:
                            ut = upool.tile([128, 2048], dt.float16, tag="qu")
                            nc.vector.tensor_tensor(ut[:], upr[0], yt[:],
                                                    op=Alu.add)
                            uap = ut[:]
                        nc.vector.tensor_scalar(
                            wts(spk)[:, :, t],
                            uap.rearrange("p (w s) -> p w s", w=8, s=256),
                            float(2.0 ** t), None, op0=Alu.is_ge)
                        if t < T - 1:
                            mk = upool.tile([128, 2048], dt.float16, tag="qu")
                            nc.vector.tensor_scalar(
                                mk[:], uap, float(2.0 ** t), None,
                                op0=Alu.is_lt)
                            wt_ = upool.tile([128, 2048], dt.float16,
                                             tag="qu")
                            nc.vector.tensor_tensor(wt_[:], uap, mk[:],
                                                    op=Alu.mult)
                            upr[0] = wt_[:]

                def gather_half(half):
                    at_sb = athfp.tile([8, 4096], dt.float16, tag="athf")
                    attn_half[half] = at_sb
                    ks8v = ks8[:].rearrange("p (w hs) -> p w hs", w=8, hs=1024)
                    vs8v = vs8[:].rearrange("p (w hs) -> p w hs", w=8, hs=1024)
                    sc8v = scid8[:].rearrange("p (b c) -> p b c", b=64, c=128)
                    hs = slice(half * 512, half * 512 + 512)
                    for w in range(NUM_WINS):
                        kag = psKV.tile([128, 512], dt.float32, tag="kag")
                        vag = psKV.tile([128, 512], dt.float32, tag="vag")
                        for ps, src in ((kag, ks8v), (vag, vs8v)):
                            for pr in range(4):
                                lhs = sc8v[:, w * 8 + 2 * pr:
                                           w * 8 + 2 * pr + 2]
                                rhs = src[:, 2 * pr:2 * pr + 2, hs]
                                nc.tensor.matmul(ps[:], lhs, rhs,
                                                 start=(pr == 0),
                                                 stop=(pr == 3),
                                                 perf_mode=DR)
                        nc.scalar.activation(
                            vag8[:, w * 1024 + half * 512:
                                 w * 1024 + half * 512 + 512],
                            vag[:], Act.Copy)
                        qk = qkp.tile([128, 512], dt.bfloat16, tag="qk")
                        nc.vector.tensor_tensor(
                            qk[:], qs8[:, w * 1024 + half * 512:
                                        w * 1024 + half * 512 + 512],
                            kag[:], op=Alu.mult)
                        if debug and half == 0 and w == 0:
                            nc.sync.dma_start(dbg["qk"][:], qk[:])
                        at = psAt.tile([8, 512], dt.float32, tag="at")
                        for ts_ in range(2):
                            t = 2 * half + ts_
                            nc.tensor.matmul(
                                at[:, ts_ * 256:(ts_ + 1) * 256],
                                amats_sb[:, t * 8:t * 8 + 8],
                                qk[:, ts_ * 256:(ts_ + 1) * 256],
                                start=True, stop=True)
                        nc.scalar.activation(
                            at_sb[:, w * 512:(w + 1) * 512], at[:], Act.Copy)

                def attn_lif(t):
                    half, ts_ = t // 2, t % 2
                    ysl = attn_half[half][:].rearrange(
                        "p (w u s) -> p w u s", w=8, u=2, s=256)[:, :, ts_]
                    if t == 0:
                        uap = ysl
                    else:
                        ut = atup.tile([8, 2048], dt.float16, tag="atu")
                        nc.vector.tensor_tensor(
                            ut[:].rearrange("p (w s) -> p w s", w=8, s=256),
                            uat_prev[0], ysl, op=Alu.add)
                        uap = ut[:].rearrange("p (w s) -> p w s", w=8, s=256)
                    at_s = atsp.tile([8, 2048], dt.float8e4, tag="ats")
                    nc.vector.tensor_scalar(
                        at_s[:].rearrange("p (w s) -> p w s", w=8, s=256),
                        uap, P2[t], None, op0=Alu.is_ge)
                    if t < T - 1:
                        wt_ = atwp.tile([8, 2048], dt.float16, tag="atu")
                        nc.vector.scalar_tensor_tensor(
                            wt_[:].rearrange("p (w s) -> p w s", w=8, s=256),
                            uap, P2[t], uap, op0=Alu.is_lt, op1=Alu.mult)
                        uat_prev[0] = wt_[:].rearrange(
                            "p (w s) -> p w s", w=8, s=256)
                    return at_s

                def z_stage(t, at_s):
                    p16 = p16_box[0]
                    yz = yzp.tile([128, 2048], dt.bfloat16, tag="yz")
                    at_v = at_s[:].rearrange("p (w s) -> p w s", w=8, s=256)
                    for wp in range(4):
                        ex = psEx.tile([128, 512], dt.float32, tag="ex")
                        nc.tensor.matmul(ex[:], emat_sb[:],
                                         at_v[:, 2 * wp:2 * wp + 2],
                                         start=True, stop=True)
                        vsl = wts(vag8)[:, 2 * wp:2 * wp + 2, t]
                        ydst = yz[:, wp * 512:(wp + 1) * 512].rearrange(
                            "p (w s) -> p w s", w=2, s=256)
                        nc.vector.scalar_tensor_tensor(
                            ydst, ex[:].rearrange("p (w s) -> p w s",
                                                  w=2, s=256),
                            0.5 * P2[t], vsl, op0=Alu.mult, op1=Alu.mult)
                    if t == 0:
                        uap = yz[:]
                    else:
                        ut = zup.tile([128, 2048], dt.bfloat16, tag="yz")
                        nc.vector.tensor_tensor(ut[:], uz_prev[0], yz[:],
                                                op=Alu.add)
                        uap = ut[:]
                    zt = ztp.tile([128, 2048], dt.bfloat16, tag="zt")
                    nc.vector.tensor_scalar(zt[:], uap, P2[t], None,
                                            op0=Alu.is_ge)
                    if t < T - 1:
                        mkz = zwp.tile([128, 2048], dt.bfloat16, tag="yz")
                        nc.vector.tensor_scalar(mkz[:], uap, P2[t], None,
                                                op0=Alu.is_lt)
                        wt_ = zwp.tile([128, 2048], dt.bfloat16, tag="yz")
                        nc.vector.tensor_tensor(wt_[:], uap, mkz[:],
                                                op=Alu.mult)
                        uz_prev[0] = wt_[:]
                    if debug:
                        nc.sync.dma_start(dbg["z"][t], zt[:])
                    ztv = zt[:].rearrange("p (w dh dw) -> p w dh dw",
                                          w=8, dh=16, dw=16)
                    for g in range(2):
                        for hh in range(2):
                            w0 = 4 * g + 2 * hh
                            rhs = ztv[:, w0:w0 + 2].rearrange(
                                "p ww dh dw -> p dh ww dw")
                            pp = psP.tile([128, 512], dt.float32, tag="pp")
                            nc.tensor.matmul(pp[:], pwT_sb[:, 0:128], rhs,
                                             start=True, stop=True)
                            slot = (t * 2 + g) * 2 + hh
                            dst = p16[:, t * 2048 + g * 1024 + hh * 512:
                                      t * 2048 + g * 1024 + hh * 512 + 512]
                            nc.scalar.activation(
                                dst, pp[:], Act.Copy,
                                accum_out=psums[:, slot:slot + 1])
                            pscr = scr512.tile([128, 512], dt.float32,
                                               tag="sq512")
                            nc.scalar.activation(
                                pscr[:], pp[:], Act.Square,
                                accum_out=psq[:, slot:slot + 1])

                lif_qk(0)
                lif_qk(1)
                gather_half(0)
                if debug:
                    nc.sync.dma_start(dbg["vag_e"][:],
                                      wts(vag8)[:, :, 0:2])
                    nc.sync.dma_start(dbg["attn_e"][:], attn_half[0][:])
                lif_qk(2)
                lif_qk(3)
                if debug:
                    nc.sync.dma_start(dbg["qlin"][:], qlin[:])
                    nc.sync.dma_start(dbg["klin"][:], klin[:])
                p16_box[0] = p16g
                z_stage(0, attn_lif(0))
                gather_half(1)
                z_stage(1, attn_lif(1))
                for t in (2, 3):
                    z_stage(t, attn_lif(t))

                if debug:
                    nc.sync.dma_start(dbg["vag"][:], vag8[:])
                    nc.sync.dma_start(dbg["scid"][:], scid8[:])
                    nc.sync.dma_start(dbg["qs"][:], qs8[:])
                    nc.sync.dma_start(dbg["ks"][:], ks8[:])
                    nc.sync.dma_start(dbg["vs"][:], vs8[:])
                    nc.sync.dma_start(dbg["attn"][:, 0:4096], attn_half[0][:])
                    nc.sync.dma_start(dbg["attn"][:, 4096:8192],
                                      attn_half[1][:])
                    nc.sync.dma_start(dbg["p"][:], p16_box[0][:])

                # ============ STAGE D: AR2 + final affine + out DMA ========
                pay2 = smallp.tile([128, 8], dt.float32, tag="pay2")
                nc.vector.tensor_reduce(pay2[:, 0:1], psums[:],
                                        axis=mybir.AxisListType.X, op=Alu.add)
                nc.vector.tensor_reduce(pay2[:, 1:2], psq[:],
                                        axis=mybir.AxisListType.X, op=Alu.add)
                nc.gpsimd.memset(pay2[:, 2:8], 0.0)
                cc2_in = dramp.tile([128, 8], dt.float32, tag="cc2_in")
                cc2_out = dramp.tile([128, 8], dt.float32, tag="cc2_out")
                nc.sync.dma_start(cc2_in[:], pay2[:])
                nc.gpsimd.collective_compute(
                    "AllReduce", Alu.add, replica_groups=RG,
                    ins=[cc2_in.opt()], outs=[cc2_out.opt()])
                S2 = smallp.tile([128, 8], dt.float32, tag="S2")
                nc.sync.dma_start(S2[:], cc2_out[:])

                meanp = smallp.tile([128, 1], dt.float32, tag="meanp")
                nc.vector.tensor_scalar(meanp[:], S2[:, 0:1], 1.0 / NTOT,
                                        None, op0=Alu.mult)
                varp = smallp.tile([128, 1], dt.float32, tag="varp")
                nc.vector.tensor_scalar(varp[:], S2[:, 1:2], 1.0 / NTOT,
                                        None, op0=Alu.mult)
                msqp = smallp.tile([128, 1], dt.float32, tag="msqp")
                nc.vector.tensor_tensor(msqp[:], meanp[:], meanp[:],
                                        op=Alu.mult)
                nc.vector.tensor_tensor(varp[:], varp[:], msqp[:],
                                        op=Alu.subtract)
                nc.vector.tensor_scalar(varp[:], varp[:], EPS, None,
                                        op0=Alu.add)
                nc.scalar.sqrt(varp[:], varp[:])
                rstdp = smallp.tile([128, 1], dt.float32, tag="rstdp")
                nc.vector.reciprocal(rstdp[:], varp[:])
                ap_ = smallp.tile([128, 1], dt.float32, tag="ap_")
                nc.vector.tensor_tensor(ap_[:], gb_sb[:, 6:7], rstdp[:],
                                        op=Alu.mult)
                bp_ = smallp.tile([128, 1], dt.float32, tag="bp_")
                nc.vector.tensor_tensor(bp_[:], ap_[:], meanp[:], op=Alu.mult)
                nc.vector.tensor_tensor(bp_[:], gb_sb[:, 7:8], bp_[:],
                                        op=Alu.subtract)

                with tc.tile_pool(name="outp", bufs=2) as outp:
                    for t in range(T):
                        for g in range(2):
                            oft = outp.tile([128, 1024], dt.bfloat16,
                                            tag="of")
                            of = oft[:]
                            src = p16_box[0][:, t * 2048 + g * 1024:
                                             t * 2048 + (g + 1) * 1024]
                            if (t * 2 + g) % 2 == 0:
                                nc.vector.tensor_scalar(
                                    of, src, ap_[:], bp_[:],
                                    op0=Alu.mult, op1=Alu.add)
                            else:
                                nc.scalar.activation(
                                    of, src, Act.Identity,
                                    bias=bp_[:], scale=ap_[:])
                            eng = (nc.sync, nc.scalar)[(t * 2 + g) % 2]
                            eng.dma_start(out_d[t, g], of)
            latestack.close()

    nc.compile()
    return nc


def _host_inputs(x, qw, q_gamma, q_beta, kw, k_gamma, k_beta,
                 v_gamma, v_beta, pw, p_gamma, p_beta):
    """Build the 8 per-core input dicts."""
    f32 = np.float32
    bf16 = ml_dtypes.bfloat16
    f8 = ml_dtypes.float8_e4m3
    x = np.ascontiguousarray(x, f32)
    qw = np.asarray(qw, f32)
    kw = np.asarray(kw, f32)
    pw = np.asarray(pw, f32)

    kd = (qw[:, :, 0] + qw[:, :, 2]).sum((-1, -2))  # [O, I]
    qw_eff = qw.copy()
    qw_eff[:, :, 1, 1, 1] -= THETA * kd
    w27 = qw_eff.reshape(128, 128, 27).transpose(1, 2, 0).reshape(128, 27 * 128)
    w27 = np.ascontiguousarray(w27, f32)
    kwT = np.ascontiguousarray(kw.T, f32)
    pwT = pw.T  # [i, o]
    pw_hi = pwT.astype(bf16)
    pw_lo = (pwT - pw_hi.astype(f32)).astype(bf16)
    pwT2 = np.stack([pw_hi, pw_lo])

    gb = np.stack([q_gamma, q_beta, k_gamma, k_beta, v_gamma, v_beta,
                   p_gamma, p_beta], axis=1).astype(f32)
    ident = (0.25 * np.eye(128)).astype(bf16)
    amats = np.zeros((128, 32), bf16)
    for c in range(128):
        for t in range(T):
            amats[c, t * 8 + c // 16] = 0.5 * (2.0 ** t)
    emat8 = np.zeros((8, 128), f8)
    for c in range(128):
        emat8[c // 16, c] = 1.0

    # x windowed: [t, b, c, wt, dt, wh, dh, ww, dw]
    xw = x.reshape(T, B, C, 2, 4, 2, LH, 2, LW)

    in_maps = []
    for core in range(8):
        b, j = core // 4, core % 4
        xconv = np.zeros((T, 2, 3, 128, 34, 34), f32)
        for g in range(2):
            for dd in range(3):
                d = j + 4 * g + dd - 1
                if 0 <= d < D:
                    xconv[:, g, dd, :, 1:33, 1:33] = x[:, b, :, d]
        # xv[c, w, t, s]: w = wt*4+wh*2+ww, s = dh*16+dw, dt=j
        xvw = xw[:, b, :, :, j]  # [t, c, wt, wh, dh, ww, dw]
        xvw = xvw.transpose(1, 2, 3, 5, 0, 4, 6)  # [c, wt, wh, ww, t, dh, dw]
        xv = xvw.reshape(C, NUM_WINS, T, 256)
        bm16 = np.zeros((128, 16), f32)
        bm16[:, b * 8:(b + 1) * 8] = 1.0
        romats = np.zeros((8, 256), f32)
        for k in range(8):
            for w in range(8):
                for w2 in range(8):
                    if k == w:
                        romats[k, w * 8 + w2] = 1.0
                    if k == w2:
                        romats[k, 64 + w * 8 + w2] = 1.0
        romats[:, 128:256] = 1.0
        in_maps.append({
            "xconv": np.ascontiguousarray(xconv.reshape(T, 2, 3, 128, 34 * 34)),
            "xv": np.ascontiguousarray(xv),
            "w27": w27, "kwT": kwT, "pwT2": pwT2, "gb": gb,
            "identw": ident, "amats": amats, "emat8": emat8, "bmask16": bm16,
            "romats": romats,
        })
    return in_maps


def kernel(**inputs):
    from concourse.bass_utils import run_bass_kernel_spmd

    key = ("dbg" if DEBUG else "plain")
    if key not in _COMPILED:
        _COMPILED[key] = _build(DEBUG)
    nc = _COMPILED[key]

    in_maps = _host_inputs(**inputs)
    res = run_bass_kernel_spmd(nc, in_maps, core_ids=list(range(8)))
    kernel.last_results = res

    full = np.empty((T, B, C, D, H, W), np.float32)
    for core in range(8):
        b, j = core // 4, core % 4
        oc = np.asarray(res.results[core]["out"], dtype=np.float32)
        for g in range(2):
            full[:, b, :, j + 4 * g] = oc[:, g].reshape(T, C, H, W)
    return full


# revision 44
# speedup vs baseline: 1.0236x; 1.0156x over previous
"""Trainium2 Bass kernel for nn_BiSDA (spiking bi-directional sparse attention).

v2 strategy (8 NeuronCores, single SPMD launch), core c = b*4 + j:
  - Layout: all big per-core tensors are [128, 8192] with free index
    w*1024 + t*256 + s  (window-major, s = dh*16+dw within window, dt=j).
  - v-chain (lif(bn(x))) depends only on x: its BN stats AllReduce runs at
    kernel start (also warming the collective path) and the whole v-LIF
    executes on DVE underneath the conv (PE-bound) phase.
  - Conv psum evacuation runs on ACT (Copy + accum_out sum stats); squares
    run as tensor_tensor_reduce on DVE. No DVE copies.
  - Gather-mean over routed windows: fp8e4 DoubleRow matmuls (two stacked
    128-contractions per instruction, 0.5 cyc/row) with mask-scaled
    0.25-identity lhs; spikes are written directly as fp8 (exact).
  - Attention dot: amat_t lhs columns carry 0.5*2^t so the attention LIF
    runs in scaled form (U_t = 2^t u_t), bit-exact in fp16; the z LIF is
    likewise scaled and exact in bf16.
  - q/k LIF stay fp32: q on DVE, k u/W-updates on Pool (k spike on DVE).
  - Final projection: exact bf16 hi/lo split of pw against binary spikes;
    stats via ACT accum; second AllReduce; affine + streamed output DMA.
"""

import os
import sys

import numpy as np

sys.path.insert(0, "/opt/trn_rl_repo")

import ml_dtypes  # noqa: E402

T, B, C = 4, 2, 128
D, H, W = 8, 32, 32
NUM_WINS = 8
LH, LW = 16, 16
NUM_HEADS, HEAD_DIM = 8, 16
THETA = 0.7
EPS = 1e-5
NTOT = float(T * B * D * H * W)
REGION_N = float(T * 4 * LH * LW)

DEBUG = bool(int(os.environ.get("BISDA_DEBUG", "0")))

_COMPILED = {}


def _build(debug):
    import concourse.bacc as bacc
    import concourse.mybir as mybir
    from concourse import tile

    dt = mybir.dt
    Alu = mybir.AluOpType
    Act = mybir.ActivationFunctionType
    DR = mybir.MatmulPerfMode.DoubleRow

    nc = bacc.Bacc("TRN2", target_bir_lowering=False, debug=False,
                   enable_asserts=False, num_devices=8)

    # ---------------- DRAM I/O ----------------
    xconv = nc.dram_tensor("xconv", [T, 2, 3, 128, 34 * 34], dt.float32r,
                           kind="ExternalInput")
    xv = nc.dram_tensor("xv", [128, NUM_WINS, T, 256], dt.float32,
                        kind="ExternalInput")
    w27 = nc.dram_tensor("w27", [128, 27 * 128], dt.float32r, kind="ExternalInput")
    kwT = nc.dram_tensor("kwT", [128, 128], dt.float32r, kind="ExternalInput")
    pwT2 = nc.dram_tensor("pwT2", [2, 128, 128], dt.bfloat16, kind="ExternalInput")
    # gb columns: q_gamma,q_beta,k_gamma,k_beta,v_gamma,v_beta,p_gamma,p_beta
    gb = nc.dram_tensor("gb", [128, 8], dt.float32, kind="ExternalInput")
    identw = nc.dram_tensor("identw", [128, 128], dt.bfloat16,
                            kind="ExternalInput")  # 0.25 * I
    amats = nc.dram_tensor("amats", [128, 32], dt.bfloat16,
                           kind="ExternalInput")  # col t*8+h: 0.5*2^t one-hot
    emat8 = nc.dram_tensor("emat8", [8, 128], dt.float8e4,
                           kind="ExternalInput")  # 1.0 one-hot expand
    bmask16 = nc.dram_tensor("bmask16", [128, 16], dt.float32,
                             kind="ExternalInput")
    # routing broadcast consts: cols 0:64 wsel[k,(w,w')]= (k==w);
    # 64:128 mask1[k,(w,w')] = (k==w'); 128:256 ones
    romats = nc.dram_tensor("romats", [8, 256], dt.float32,
                            kind="ExternalInput")

    out_d = nc.dram_tensor("out", [T, 2, 128, 1024], dt.bfloat16,
                           kind="ExternalOutput")
    dbg = {}
    if debug:
        dbg["qlin"] = nc.dram_tensor("dbg_qlin", [128, 8192], dt.float32,
                                     kind="ExternalOutput")
        dbg["klin"] = nc.dram_tensor("dbg_klin", [128, 8192], dt.float32,
                                     kind="ExternalOutput")
        dbg["stats"] = nc.dram_tensor("dbg_stats", [128, 40], dt.float32,
                                      kind="ExternalOutput")
        dbg["m"] = nc.dram_tensor("dbg_m", [8, 8], dt.float32,
                                  kind="ExternalOutput")
        dbg["qs"] = nc.dram_tensor("dbg_qs", [128, 8192], dt.float8e4,
                                   kind="ExternalOutput")
        dbg["ks"] = nc.dram_tensor("dbg_ks", [128, 8192], dt.float8e4,
                                   kind="ExternalOutput")
        dbg["vs"] = nc.dram_tensor("dbg_vs", [128, 8192], dt.float8e4,
                                   kind="ExternalOutput")
        dbg["attn"] = nc.dram_tensor("dbg_attn", [8, 8192], dt.float16,
                                     kind="ExternalOutput")
        dbg["z"] = nc.dram_tensor("dbg_z", [4, 128, 2048], dt.bfloat16,
                                  kind="ExternalOutput")
        dbg["p"] = nc.dram_tensor("dbg_p", [128, 8192], dt.bfloat16,
                                  kind="ExternalOutput")
        dbg["vag"] = nc.dram_tensor("dbg_vag", [128, 8192], dt.float8e4,
                                    kind="ExternalOutput")
        dbg["qk"] = nc.dram_tensor("dbg_qk", [128, 512], dt.bfloat16,
                                   kind="ExternalOutput")
        dbg["scid"] = nc.dram_tensor("dbg_scid", [128, 8192], dt.float8e4,
                                     kind="ExternalOutput")
        dbg["scid_e"] = nc.dram_tensor("dbg_scid_e", [128, 8192], dt.float8e4,
                                       kind="ExternalOutput")
        dbg["vag_e"] = nc.dram_tensor("dbg_vag_e", [128, 8, 2, 256],
                                      dt.float8e4, kind="ExternalOutput")
        dbg["attn_e"] = nc.dram_tensor("dbg_attn_e", [8, 4096], dt.float16,
                                       kind="ExternalOutput")

    RG = [[0, 1, 2, 3, 4, 5, 6, 7]]
    P2 = [1.0, 2.0, 4.0, 8.0]  # 2^t

    with tile.TileContext(nc) as tc:
        with (
            tc.tile_pool(name="const", bufs=1) as constp,
            tc.tile_pool(name="dram", bufs=1, space="DRAM") as dramp,
            tc.tile_pool(name="big", bufs=1) as bigp,
            tc.tile_pool(name="small", bufs=1) as smallp,
            tc.tile_pool(name="scr512", bufs=2) as scr512,
        ):
            # ---- constants ----
            kwT_sb = constp.tile([128, 128], dt.float32r, tag="kwT")
            nc.sync.dma_start(kwT_sb[:], kwT[:])
            pwT_sb = constp.tile([128, 256], dt.bfloat16, tag="pwT")
            nc.sync.dma_start(pwT_sb[:, 0:128], pwT2[0])
            nc.sync.dma_start(pwT_sb[:, 128:256], pwT2[1])
            gb_sb = constp.tile([128, 8], dt.float32, tag="gb")
            nc.sync.dma_start(gb_sb[:], gb[:])
            ident_sb = constp.tile([128, 128], dt.bfloat16, tag="ident")
            nc.sync.dma_start(ident_sb[:], identw[:])
            amats_sb = constp.tile([128, 32], dt.bfloat16, tag="amats")
            nc.sync.dma_start(amats_sb[:], amats[:])
            emat_sb = constp.tile([8, 128], dt.float8e4, tag="emat")
            nc.sync.dma_start(emat_sb[:], emat8[:])
            bm16_sb = constp.tile([128, 16], dt.float32, tag="bm16")
            nc.sync.dma_start(bm16_sb[:], bmask16[:])
            ro_sb = constp.tile([8, 256], dt.float32, tag="romats")
            nc.sync.dma_start(ro_sb[:], romats[:])

            # persistent big tensors
            qlin = bigp.tile([128, 8192], dt.float32, tag="qlin")
            klin = bigp.tile([128, 8192], dt.float32, tag="klin")
            vs8 = bigp.tile([128, 8192], dt.float8e4, tag="vs8")

            def wts(big):
                return big[:].rearrange("p (w t s) -> p w t s",
                                        w=8, t=4, s=256)

            def v3(tl):
                return tl[:].rearrange("p (w s) -> p w s", w=8, s=256)

            sums = smallp.tile([128, 72], dt.float32, tag="sums")
            # qsum 0:16, qsq 16:32, ksum 32:48, ksq 48:64, xsum 64:66,
            # xsq 66:68  (q/k slots: (t*2+g)*2+hh)
            qregs = smallp.tile([128, 32], dt.float32, tag="qregs")
            kregs = smallp.tile([128, 32], dt.float32, tag="kregs")
            psums = smallp.tile([128, 16], dt.float32, tag="psums")
            psq = smallp.tile([128, 16], dt.float32, tag="psq")
            aff = smallp.tile([128, 16], dt.float32, tag="aff")
            # aff cols: 0 aq_h,1 bq_h,2 ak_h,3 bk_h,4 av_h,5 bv_h,
            #           6 aq_f,7 bq_f,8 ak_f,9 bk_f

            def bn_affine(scol, gcol, St_tile, dst_half, dst_full, name):
                mean = smallp.tile([128, 1], dt.float32, tag=f"mean{name}")
                nc.vector.tensor_scalar(mean[:], St_tile[:, scol:scol + 1],
                                        1.0 / NTOT, None, op0=Alu.mult)
                var = smallp.tile([128, 1], dt.float32, tag=f"var{name}")
                nc.vector.tensor_scalar(var[:], St_tile[:, scol + 1:scol + 2],
                                        1.0 / NTOT, None, op0=Alu.mult)
                msq = smallp.tile([128, 1], dt.float32, tag=f"msq{name}")
                nc.vector.tensor_tensor(msq[:], mean[:], mean[:], op=Alu.mult)
                nc.vector.tensor_tensor(var[:], var[:], msq[:], op=Alu.subtract)
                nc.vector.tensor_scalar(var[:], var[:], EPS, None, op0=Alu.add)
                nc.scalar.sqrt(var[:], var[:])
                rstd = smallp.tile([128, 1], dt.float32, tag=f"rstd{name}")
                nc.vector.reciprocal(rstd[:], var[:])
                afull = smallp.tile([128, 1], dt.float32, tag=f"af{name}")
                nc.vector.tensor_tensor(afull[:], gb_sb[:, gcol:gcol + 1],
                                        rstd[:], op=Alu.mult)
                bfull = smallp.tile([128, 1], dt.float32, tag=f"bf{name}")
                nc.vector.tensor_tensor(bfull[:], afull[:], mean[:], op=Alu.mult)
                nc.vector.tensor_tensor(bfull[:], gb_sb[:, gcol + 1:gcol + 2],
                                        bfull[:], op=Alu.subtract)
                nc.vector.tensor_scalar(aff[:, dst_half:dst_half + 1], afull[:],
                                        0.5, None, op0=Alu.mult)
                nc.vector.tensor_scalar(aff[:, dst_half + 1:dst_half + 2],
                                        bfull[:], 0.5, None, op0=Alu.mult)
                if dst_full is not None:
                    nc.vector.tensor_copy(aff[:, dst_full:dst_full + 1], afull[:])
                    nc.vector.tensor_copy(aff[:, dst_full + 1:dst_full + 2],
                                          bfull[:])

            # ============ STAGE 0 + A: xv, x-stats, AR_v, conv ============
            with (
                tc.tile_pool(name="xvp", bufs=1) as xvp,
                tc.tile_pool(name="vlif", bufs=3) as vlifp,
                tc.tile_pool(name="xin", bufs=6) as xinp,
                tc.tile_pool(name="psA", bufs=6, space="PSUM") as psA,
                tc.tile_pool(name="psK", bufs=2, space="PSUM") as psK,
                tc.tile_pool(name="w27p", bufs=1) as w27p,
            ):
                w27_sb = w27p.tile([128, 27 * 128], dt.float32r, tag="w27")
                nc.sync.dma_start(w27_sb[:, 0:9 * 128], w27[:, 0:9 * 128])
                xpads0 = {}
                for dd in range(3):
                    for g in range(2):
                        xp = xinp.tile([128, 34 * 34], dt.float32r, tag="xpad")
                        nc.sync.dma_start(xp[:], xconv[0, g, dd])
                        xpads0[(g, dd)] = xp
                nc.sync.dma_start(w27_sb[:, 9 * 128:27 * 128],
                                  w27[:, 9 * 128:27 * 128])
                xv_sb = xvp.tile([128, 8192], dt.float32, tag="xv")

                # ---- conv ----
                for t in range(T):
                    if t == 3:
                        nc.sync.dma_start(
                            xv_sb[:],
                            xv[:].rearrange("p w t s -> p (w t s)"))
                        for q4 in range(4):
                            xsl = xv_sb[:, q4 * 2048:(q4 + 1) * 2048]
                            xsc = vlifp.tile([128, 2048], dt.float32,
                                             tag="vu")
                            nc.scalar.activation(
                                xsc[:], xsl, Act.Copy,
                                accum_out=sums[:, 64 + q4:65 + q4])
                            xsc2 = vlifp.tile([128, 2048], dt.float32,
                                              tag="vu")
                            nc.scalar.activation(
                                xsc2[:], xsl, Act.Square,
                                accum_out=sums[:, 68 + q4:69 + q4])
                        pay_v = smallp.tile([128, 8], dt.float32, tag="pay_v")
                        nc.gpsimd.memset(pay_v[:], 0.0)
                        nc.vector.tensor_reduce(
                            pay_v[:, 0:1], sums[:, 64:68],
                            axis=mybir.AxisListType.X, op=Alu.add)
                        nc.vector.tensor_reduce(
                            pay_v[:, 1:2], sums[:, 68:72],
                            axis=mybir.AxisListType.X, op=Alu.add)
                        ccv_in = dramp.tile([128, 8], dt.float32,
                                            tag="ccv_in")
                        ccv_out = dramp.tile([128, 8], dt.float32,
                                             tag="ccv_out")
                        nc.sync.dma_start(ccv_in[:], pay_v[:])
                        nc.gpsimd.collective_compute(
                            "AllReduce", Alu.add, replica_groups=RG,
                            ins=[ccv_in.opt()], outs=[ccv_out.opt()])
                        Sv = smallp.tile([128, 8], dt.float32, tag="Sv")
                        nc.sync.dma_start(Sv[:], ccv_out[:])
                        bn_affine(0, 4, Sv, 4, None, "v")
                    if t == 0:
                        xpads = xpads0
                    else:
                        xpads = {}
                        for dd in range(3):
                            for g in range(2):
                                xp = xinp.tile([128, 34 * 34], dt.float32r,
                                               tag="xpad")
                                nc.sync.dma_start(xp[:], xconv[t, g, dd])
                                xpads[(g, dd)] = xp
                    pss = {}
                    for g in range(2):
                        for hh in range(2):
                            psx = psA.tile([128, 512], dt.float32, tag="psA")
                            pss[(g, hh)] = psx
                    for tap in range(27):
                        dd, rem = divmod(tap, 9)
                        dhh, dww = divmod(rem, 3)
                        lhs = w27_sb[:, tap * 128:(tap + 1) * 128]
                        for g in range(2):
                            for hh in range(2):
                                rhs = xpads[(g, dd)][:].rearrange(
                                    "p (r c) -> p r c", r=34, c=34
                                )[:, 16 * hh + dhh:16 * hh + dhh + 16,
                                  dww:dww + 32]
                                nc.tensor.matmul(pss[(g, hh)][:], lhs, rhs,
                                                 start=(tap == 0),
                                                 stop=(tap == 26))
                    # k_lin from center planes, rhs (ww, dh, dw)
                    for g in range(2):
                        for hh in range(2):
                            slot = (t * 2 + g) * 2 + hh
                            psk = psK.tile([128, 512], dt.float32, tag="psK")
                            ctr = xpads[(g, 1)][:].rearrange(
                                "p (r c) -> p r c", r=34, c=34)
                            rhs = ctr[:, 16 * hh + 1:16 * hh + 17, 1:33]
                            rhs = rhs.rearrange(
                                "p dh (ww dw) -> p ww dh dw", ww=2, dw=16)
                            nc.tensor.matmul(psk[:], kwT_sb[:], rhs,
                                             start=True, stop=True)
                            w0 = 4 * g + 2 * hh
                            kdst = wts(klin)[:, w0:w0 + 2, t]
                            nc.scalar.activation(
                                kdst,
                                psk[:].rearrange("p (ww s) -> p ww s",
                                                 ww=2, s=256),
                                Act.Copy,
                                accum_out=sums[:, 32 + slot:33 + slot])
                            ksc = scr512.tile([128, 512], dt.float32,
                                              tag="sq512")
                            nc.scalar.activation(
                                ksc[:], psk[:], Act.Square,
                                accum_out=sums[:, 48 + slot:49 + slot])
                        rc = g * 16 + t * 4
                        nc.vector.tensor_reduce(
                            kregs[:, rc:rc + 4],
                            wts(klin)[:, 4 * g:4 * g + 4, t],
                            axis=mybir.AxisListType.X, op=Alu.add)
                    for g in range(2):
                        for hh in range(2):
                            slot = (t * 2 + g) * 2 + hh
                            w0 = 4 * g + 2 * hh
                            dstap = wts(qlin)[:, w0:w0 + 2, t].rearrange(
                                "p ww (dh dw) -> p dh ww dw", dh=16, dw=16)
                            nc.scalar.activation(
                                dstap, pss[(g, hh)][:].rearrange(
                                    "p (dh ww dw) -> p dh ww dw",
                                    ww=2, dh=16, dw=16),
                                Act.Copy,
                                accum_out=sums[:, slot:slot + 1])
                            qsc = scr512.tile([128, 512], dt.float32,
                                              tag="sq512")
                            nc.scalar.activation(
                                qsc[:], pss[(g, hh)][:], Act.Square,
                                accum_out=sums[:, 16 + slot:17 + slot])
                        rc = g * 16 + t * 4
                        nc.vector.tensor_reduce(
                            qregs[:, rc:rc + 4],
                            wts(qlin)[:, 4 * g:4 * g + 4, t],
                            axis=mybir.AxisListType.X, op=Alu.add)

                    if t == 3:
                        wv = None
                        for tv in range(T):
                            yv = vlifp.tile([128, 2048], dt.float32, tag="vu")
                            nc.scalar.activation(
                                v3(yv), wts(xv_sb)[:, :, tv],
                                Act.Identity, bias=aff[:, 5:6],
                                scale=aff[:, 4:5])
                            if tv == 0:
                                u = yv
                            else:
                                un = vlifp.tile([128, 2048], dt.float32,
                                                tag="vu")
                                nc.vector.scalar_tensor_tensor(
                                    un[:], wv[:], 0.5, yv[:],
                                    op0=Alu.mult, op1=Alu.add)
                                u = un
                            nc.vector.tensor_scalar(
                                wts(vs8)[:, :, tv], v3(u), 1.0, None,
                                op0=Alu.is_ge)
                            if tv < T - 1:
                                wn = vlifp.tile([128, 2048], dt.float32,
                                                tag="vu")
                                nc.vector.scalar_tensor_tensor(
                                    wn[:], u[:], 1.0, u[:],
                                    op0=Alu.is_lt, op1=Alu.mult)
                                wv = wn

            # ============ STAGE B: AR1 + affines + routing ============
            from contextlib import ExitStack
            latestack = ExitStack()
            latep = latestack.enter_context(tc.tile_pool(name="late", bufs=1))
            p16g = latep.tile([128, 8192], dt.bfloat16, tag="p16")
            qs8 = latep.tile([128, 8192], dt.float8e4, tag="qs8")
            ks8 = latep.tile([128, 8192], dt.float8e4, tag="ks8")
            scid8 = latep.tile([128, 8192], dt.float8e4, tag="scid8")
            vag8 = latep.tile([128, 8192], dt.float8e4, tag="vag8")

            payload = smallp.tile([128, 40], dt.float32, tag="payload")
            for col, (base, cnt) in enumerate(
                    [(0, 16), (16, 16), (32, 16), (48, 16)]):
                nc.vector.tensor_reduce(
                    payload[:, col:col + 1], sums[:, base:base + cnt],
                    axis=mybir.AxisListType.X, op=Alu.add)
            qreg8 = smallp.tile([128, 8], dt.float32, tag="qreg8")
            nc.vector.tensor_reduce(
                qreg8[:].rearrange("p (g w) -> p g w", g=2, w=4),
                qregs[:].rearrange("p (g t w) -> p g w t", g=2, t=4, w=4),
                axis=mybir.AxisListType.X, op=Alu.add)
            kreg8 = smallp.tile([128, 8], dt.float32, tag="kreg8")
            nc.vector.tensor_reduce(
                kreg8[:].rearrange("p (g w) -> p g w", g=2, w=4),
                kregs[:].rearrange("p (g t w) -> p g w t", g=2, t=4, w=4),
                axis=mybir.AxisListType.X, op=Alu.add)
            nc.vector.tensor_tensor(payload[:, 4:12], qreg8[:],
                                    bm16_sb[:, 0:8], op=Alu.mult)
            nc.vector.tensor_tensor(payload[:, 12:20], qreg8[:],
                                    bm16_sb[:, 8:16], op=Alu.mult)
            nc.vector.tensor_tensor(payload[:, 20:28], kreg8[:],
                                    bm16_sb[:, 0:8], op=Alu.mult)
            nc.vector.tensor_tensor(payload[:, 28:36], kreg8[:],
                                    bm16_sb[:, 8:16], op=Alu.mult)
            nc.gpsimd.memset(payload[:, 36:40], 0.0)

            cc1_in = dramp.tile([128, 40], dt.float32, tag="cc1_in")
            cc1_out = dramp.tile([128, 40], dt.float32, tag="cc1_out")
            nc.sync.dma_start(cc1_in[:], payload[:])
            nc.gpsimd.collective_compute(
                "AllReduce", Alu.add, replica_groups=RG,
                ins=[cc1_in.opt()], outs=[cc1_out.opt()])
            St = smallp.tile([128, 40], dt.float32, tag="St")
            nc.sync.dma_start(St[:], cc1_out[:])
            if debug:
                nc.sync.dma_start(dbg["stats"][:], St[:])

            bn_affine(0, 0, St, 0, 6, "q")
            bn_affine(2, 2, St, 2, 8, "k")
            aff2 = smallp.tile([128, 16], dt.float32, tag="aff2")
            for tt in range(4):
                for (cc, base) in ((0, 0), (2, 8)):
                    nc.vector.tensor_scalar(
                        aff2[:, base + 2 * tt:base + 2 * tt + 2],
                        aff[:, cc:cc + 2], float(2.0 ** tt), None,
                        op0=Alu.mult)

            # routing: region means -> BN -> select b -> a_r -> top4 mask
            qr16 = smallp.tile([128, 16], dt.float32, tag="qr16")
            nc.vector.tensor_scalar(qr16[:], St[:, 4:20], 1.0 / REGION_N, None,
                                    op0=Alu.mult)
            nc.scalar.activation(qr16[:], qr16[:], Act.Identity,
                                 bias=aff[:, 7:8], scale=aff[:, 6:7])
            kr16 = smallp.tile([128, 16], dt.float32, tag="kr16")
            nc.vector.tensor_scalar(kr16[:], St[:, 20:36], 1.0 / REGION_N, None,
                                    op0=Alu.mult)
            nc.scalar.activation(kr16[:], kr16[:], Act.Identity,
                                 bias=aff[:, 9:10], scale=aff[:, 8:9])
            tmp8 = smallp.tile([128, 8], dt.float32, tag="tmp8")
            nc.vector.scalar_tensor_tensor(
                tmp8[:], qr16[:, 0:8], bm16_sb[:, 0:1], qr16[:, 8:16],
                op0=Alu.mult, op1=Alu.bypass)
            nc.vector.scalar_tensor_tensor(
                tmp8[:], qr16[:, 8:16], bm16_sb[:, 8:9], tmp8[:],
                op0=Alu.mult, op1=Alu.add)
            tmpk8 = smallp.tile([128, 8], dt.float32, tag="tmpk8")
            nc.vector.scalar_tensor_tensor(
                tmpk8[:], kr16[:, 0:8], bm16_sb[:, 0:1], kr16[:, 8:16],
                op0=Alu.mult, op1=Alu.bypass)
            nc.vector.scalar_tensor_tensor(
                tmpk8[:], kr16[:, 8:16], bm16_sb[:, 8:9], tmpk8[:],
                op0=Alu.mult, op1=Alu.add)
            with tc.tile_pool(name="psB", bufs=1, space="PSUM") as psB:
                ar_ps = psB.tile([8, 8], dt.float32, tag="ar")
                nc.tensor.matmul(ar_ps[:], tmp8[:], tmpk8[:], start=True,
                                 stop=True)
                ar = smallp.tile([8, 8], dt.float32, tag="arsb")
                nc.vector.tensor_copy(ar[:], ar_ps[:])
            srt = smallp.tile([8, 8], dt.float32, tag="srt")
            nc.vector.max(srt[:], ar[:])
            msel = smallp.tile([8, 8], dt.float32, tag="msel")
            nc.vector.tensor_scalar(msel[:], ar[:], srt[:, 3:4], None,
                                    op0=Alu.is_ge)
            if debug:
                nc.sync.dma_start(dbg["m"][:], msel[:])
            with tc.tile_pool(name="psB2", bufs=1, space="PSUM") as psB2:
                o1 = psB2.tile([8, 64], dt.float32, tag="o1")
                nc.tensor.matmul(o1[:], msel[:], ro_sb[:, 0:64],
                                 start=True, stop=True)
                o1m = smallp.tile([8, 64], dt.float32, tag="o1m")
                nc.vector.tensor_tensor(o1m[:], o1[:], ro_sb[:, 64:128],
                                        op=Alu.mult)
                mb_ps = psB2.tile([128, 64], dt.float32, tag="mbps")
                nc.tensor.matmul(mb_ps[:], ro_sb[:, 128:256], o1m[:],
                                 start=True, stop=True)
                mbc = smallp.tile([128, 64], dt.float32, tag="mbc")
                nc.vector.tensor_copy(mbc[:], mb_ps[:])
            # scid8[c, (w*8+w')*128 + c'] = 0.25*mask (ident_sb = 0.25*I)
            for w in range(8):
                for w2 in range(8):
                    i = w * 8 + w2
                    if w % 2 == 0:
                        nc.vector.tensor_scalar(
                            scid8[:, i * 128:(i + 1) * 128], ident_sb[:],
                            mbc[:, i:i + 1], None, op0=Alu.mult)
                    else:
                        nc.scalar.activation(
                            scid8[:, i * 128:(i + 1) * 128], ident_sb[:],
                            Act.Identity, scale=mbc[:, i:i + 1])
            tc.strict_bb_all_engine_barrier()
            if debug:
                nc.sync.dma_start(dbg["scid_e"][:], scid8[:])

            # ============ STAGE C: pipelined LIF + gather + attn + out ======
            attn_half = {}
            uq_prev = [None]
            uk_prev = [None]
            uat_prev = [None]
            uz_prev = [None]
            p16_box = [None]

            from contextlib import ExitStack
            with ExitStack() as cstack:
                psKV = cstack.enter_context(
                    tc.tile_pool(name="psKV", bufs=2, space="PSUM"))
                psAt = cstack.enter_context(
                    tc.tile_pool(name="psAt", bufs=1, space="PSUM"))
                psEx = cstack.enter_context(
                    tc.tile_pool(name="psEx", bufs=1, space="PSUM"))
                psP = cstack.enter_context(
                    tc.tile_pool(name="psP", bufs=2, space="PSUM"))
                athfp = cstack.enter_context(tc.tile_pool(name="athf", bufs=1))
                atup = cstack.enter_context(tc.tile_pool(name="atuw", bufs=3))
                atwp = atup
                atsp = cstack.enter_context(tc.tile_pool(name="ats", bufs=2))
                qkp = cstack.enter_context(tc.tile_pool(name="qkp", bufs=2))
                yzp = cstack.enter_context(tc.tile_pool(name="zscr", bufs=3))
                zup = yzp
                zwp = yzp
                ztp = cstack.enter_context(tc.tile_pool(name="ztp", bufs=2))
                qlifp = cstack.enter_context(tc.tile_pool(name="lif", bufs=6))
                klifp = qlifp
                def lif_qk(t):
                    # scaled fp16 LIF: U_t = 2^t u_t = W_{t-1} + Y_t,
                    # Y_t = 2^t*(a_h*lin + b_h); spike U>=2^t; W = U*[U<2^t]
                    for (lin, spk, base, upool, upr) in (
                            (qlin, qs8, 0, qlifp, uq_prev),
                            (klin, ks8, 8, klifp, uk_prev)):
                        ysl = wts(lin)[:, :, t]
                        yt = upool.tile([128, 2048], dt.float16, tag="qu")
                        nc.scalar.activation(
                            v3(yt), ysl, Act.Identity,
                            bias=aff2[:, base + 2 * t + 1:base + 2 * t + 2],
                            scale=aff2[:, base + 2 * t:base + 2 * t + 1])
                        if t == 0:
                            uap = yt[:]
                        else:
                            ut = upool.tile([128, 2048], dt.float16, tag="qu")
                            nc.vector.tensor_tensor(ut[:], upr[0], yt[:],
                                                    op=Alu.add)
                            uap = ut[:]
                        nc.vector.tensor_scalar(
                            wts(spk)[:, :, t],
                            uap.rearrange("p (w s) -> p w s", w=8, s=256),
                            float(2.0 ** t), None, op0=Alu.is_ge)
                        if t < T - 1:
                            mk = upool.tile([128, 2048], dt.float16, tag="qu")
                            nc.vector.tensor_scalar(
                                mk[:], uap, float(2.0 ** t), None,
                                op0=Alu.is_lt)
                            wt_ = upool.tile([128, 2048], dt.float16,
                                             tag="qu")
                            nc.vector.tensor_tensor(wt_[:], uap, mk[:],
                                                    op=Alu.mult)
                            upr[0] = wt_[:]

                def gather_half(half):
                    at_sb = athfp.tile([8, 4096], dt.float16, tag="athf")
                    attn_half[half] = at_sb
                    ks8v = ks8[:].rearrange("p (w hs) -> p w hs", w=8, hs=1024)
                    vs8v = vs8[:].rearrange("p (w hs) -> p w hs", w=8, hs=1024)
                    sc8v = scid8[:].rearrange("p (b c) -> p b c", b=64, c=128)
                    hs = slice(half * 512, half * 512 + 512)
                    for w in range(NUM_WINS):
                        kag = psKV.tile([128, 512], dt.float32, tag="kag")
                        vag = psKV.tile([128, 512], dt.float32, tag="vag")
                        for ps, src in ((kag, ks8v), (vag, vs8v)):
                            for pr in range(4):
                                lhs = sc8v[:, w * 8 + 2 * pr:
                                           w * 8 + 2 * pr + 2]
                                rhs = src[:, 2 * pr:2 * pr + 2, hs]
                                nc.tensor.matmul(ps[:], lhs, rhs,
                                                 start=(pr == 0),
                                                 stop=(pr == 3),
                                                 perf_mode=DR)
                        nc.scalar.activation(
                            vag8[:, w * 1024 + half * 512:
                                 w * 1024 + half * 512 + 512],
                            vag[:], Act.Copy)
                        qk = qkp.tile([128, 512], dt.bfloat16, tag="qk")
                        nc.vector.tensor_tensor(
                            qk[:], qs8[:, w * 1024 + half * 512:
                                        w * 1024 + half * 512 + 512],
                            kag[:], op=Alu.mult)
                        if debug and half == 0 and w == 0:
                            nc.sync.dma_start(dbg["qk"][:], qk[:])
                        at = psAt.tile([8, 512], dt.float32, tag="at")
                        for ts_ in range(2):
                            t = 2 * half + ts_
                            nc.tensor.matmul(
                                at[:, ts_ * 256:(ts_ + 1) * 256],
                                amats_sb[:, t * 8:t * 8 + 8],
                                qk[:, ts_ * 256:(ts_ + 1) * 256],
                                start=True, stop=True)
                        nc.scalar.activation(
                            at_sb[:, w * 512:(w + 1) * 512], at[:], Act.Copy)

                def attn_lif(t):
                    half, ts_ = t // 2, t % 2
                    ysl = attn_half[half][:].rearrange(
                        "p (w u s) -> p w u s", w=8, u=2, s=256)[:, :, ts_]
                    if t == 0:
                        uap = ysl
                    else:
                        ut = atup.tile([8, 2048], dt.float16, tag="atu")
                        nc.vector.tensor_tensor(
                            ut[:].rearrange("p (w s) -> p w s", w=8, s=256),
                            uat_prev[0], ysl, op=Alu.add)
                        uap = ut[:].rearrange("p (w s) -> p w s", w=8, s=256)
                    at_s = atsp.tile([8, 2048], dt.float8e4, tag="ats")
                    nc.vector.tensor_scalar(
                        at_s[:].rearrange("p (w s) -> p w s", w=8, s=256),
                        uap, P2[t], None, op0=Alu.is_ge)
                    if t < T - 1:
                        wt_ = atwp.tile([8, 2048], dt.float16, tag="atu")
                        nc.vector.scalar_tensor_tensor(
                            wt_[:].rearrange("p (w s) -> p w s", w=8, s=256),
                            uap, P2[t], uap, op0=Alu.is_lt, op1=Alu.mult)
                        uat_prev[0] = wt_[:].rearrange(
                            "p (w s) -> p w s", w=8, s=256)
                    return at_s

                def z_stage(t, at_s):
                    p16 = p16_box[0]
                    yz = yzp.tile([128, 2048], dt.bfloat16, tag="yz")
                    at_v = at_s[:].rearrange("p (w s) -> p w s", w=8, s=256)
                    for wp in range(4):
                        ex = psEx.tile([128, 512], dt.float32, tag="ex")
                        nc.tensor.matmul(ex[:], emat_sb[:],
                                         at_v[:, 2 * wp:2 * wp + 2],
                                         start=True, stop=True)
                        vsl = wts(vag8)[:, 2 * wp:2 * wp + 2, t]
                        ydst = yz[:, wp * 512:(wp + 1) * 512].rearrange(
                            "p (w s) -> p w s", w=2, s=256)
                        nc.vector.scalar_tensor_tensor(
                            ydst, ex[:].rearrange("p (w s) -> p w s",
                                                  w=2, s=256),
                            0.5 * P2[t], vsl, op0=Alu.mult, op1=Alu.mult)
                    if t == 0:
                        uap = yz[:]
                    else:
                        ut = zup.tile([128, 2048], dt.bfloat16, tag="yz")
                        nc.vector.tensor_tensor(ut[:], uz_prev[0], yz[:],
                                                op=Alu.add)
                        uap = ut[:]
                    zt = ztp.tile([128, 2048], dt.bfloat16, tag="zt")
                    nc.vector.tensor_scalar(zt[:], uap, P2[t], None,
                                            op0=Alu.is_ge)
                    if t < T - 1:
                        mkz = zwp.tile([128, 2048], dt.bfloat16, tag="yz")
                        nc.vector.tensor_scalar(mkz[:], uap, P2[t], None,
                                                op0=Alu.is_lt)
                        wt_ = zwp.tile([128, 2048], dt.bfloat16, tag="yz")
                        nc.vector.tensor_tensor(wt_[:], uap, mkz[:],
                                                op=Alu.mult)
                        uz_prev[0] = wt_[:]
                    if debug:
                        nc.sync.dma_start(dbg["z"][t], zt[:])
                    ztv = zt[:].rearrange("p (w dh dw) -> p w dh dw",
                                          w=8, dh=16, dw=16)
                    for g in range(2):
                        for hh in range(2):
                            w0 = 4 * g + 2 * hh
                            rhs = ztv[:, w0:w0 + 2].rearrange(
                                "p ww dh dw -> p dh ww dw")
                            pp = psP.tile([128, 512], dt.float32, tag="pp")
                            nc.tensor.matmul(pp[:], pwT_sb[:, 0:128], rhs,
                                             start=True, stop=True)
                            slot = (t * 2 + g) * 2 + hh
                            dst = p16[:, t * 2048 + g * 1024 + hh * 512:
                                      t * 2048 + g * 1024 + hh * 512 + 512]
                            nc.scalar.activation(
                                dst, pp[:], Act.Copy,
                                accum_out=psums[:, slot:slot + 1])
                            pscr = scr512.tile([128, 512], dt.float32,
                                               tag="sq512")
                            nc.scalar.activation(
                                pscr[:], pp[:], Act.Square,
                                accum_out=psq[:, slot:slot + 1])

                lif_qk(0)
                lif_qk(1)
                gather_half(0)
                if debug:
                    nc.sync.dma_start(dbg["vag_e"][:],
                                      wts(vag8)[:, :, 0:2])
                    nc.sync.dma_start(dbg["attn_e"][:], attn_half[0][:])
                lif_qk(2)
                lif_qk(3)
                if debug:
                    nc.sync.dma_start(dbg["qlin"][:], qlin[:])
                    nc.sync.dma_start(dbg["klin"][:], klin[:])
                p16_box[0] = p16g
                z_stage(0, attn_lif(0))
                gather_half(1)
                z_stage(1, attn_lif(1))
                for t in (2, 3):
                    z_stage(t, attn_lif(t))

                if debug:
                    nc.sync.dma_start(dbg["vag"][:], vag8[:])
                    nc.sync.dma_start(dbg["scid"][:], scid8[:])
                    nc.sync.dma_start(dbg["qs"][:], qs8[:])
                    nc.sync.dma_start(dbg["ks"][:], ks8[:])
                    nc.sync.dma_start(dbg["vs"][:], vs8[:])
                    nc.sync.dma_start(dbg["attn"][:, 0:4096], attn_half[0][:])
                    nc.sync.dma_start(dbg["attn"][:, 4096:8192],
                                      attn_half[1][:])
                    nc.sync.dma_start(dbg["p"][:], p16_box[0][:])

                # ============ STAGE D: AR2 + final affine + out DMA ========
                pay2 = smallp.tile([128, 8], dt.float32, tag="pay2")
                nc.vector.tensor_reduce(pay2[:, 0:1], psums[:],
                                        axis=mybir.AxisListType.X, op=Alu.add)
                nc.vector.tensor_reduce(pay2[:, 1:2], psq[:],
                                        axis=mybir.AxisListType.X, op=Alu.add)
                nc.gpsimd.memset(pay2[:, 2:8], 0.0)
                cc2_in = dramp.tile([128, 8], dt.float32, tag="cc2_in")
                cc2_out = dramp.tile([128, 8], dt.float32, tag="cc2_out")
                nc.sync.dma_start(cc2_in[:], pay2[:])
                nc.gpsimd.collective_compute(
                    "AllReduce", Alu.add, replica_groups=RG,
                    ins=[cc2_in.opt()], outs=[cc2_out.opt()])
                S2 = smallp.tile([128, 8], dt.float32, tag="S2")
                nc.sync.dma_start(S2[:], cc2_out[:])

                meanp = smallp.tile([128, 1], dt.float32, tag="meanp")
                nc.vector.tensor_scalar(meanp[:], S2[:, 0:1], 1.0 / NTOT,
                                        None, op0=Alu.mult)
                varp = smallp.tile([128, 1], dt.float32, tag="varp")
                nc.vector.tensor_scalar(varp[:], S2[:, 1:2], 1.0 / NTOT,
                                        None, op0=Alu.mult)
                msqp = smallp.tile([128, 1], dt.float32, tag="msqp")
                nc.vector.tensor_tensor(msqp[:], meanp[:], meanp[:],
                                        op=Alu.mult)
                nc.vector.tensor_tensor(varp[:], varp[:], msqp[:],
                                        op=Alu.subtract)
                nc.vector.tensor_scalar(varp[:], varp[:], EPS, None,
                                        op0=Alu.add)
                nc.scalar.sqrt(varp[:], varp[:])
                rstdp = smallp.tile([128, 1], dt.float32, tag="rstdp")
                nc.vector.reciprocal(rstdp[:], varp[:])
                ap_ = smallp.tile([128, 1], dt.float32, tag="ap_")
                nc.vector.tensor_tensor(ap_[:], gb_sb[:, 6:7], rstdp[:],
                                        op=Alu.mult)
                bp_ = smallp.tile([128, 1], dt.float32, tag="bp_")
                nc.vector.tensor_tensor(bp_[:], ap_[:], meanp[:], op=Alu.mult)
                nc.vector.tensor_tensor(bp_[:], gb_sb[:, 7:8], bp_[:],
                                        op=Alu.subtract)

                with tc.tile_pool(name="outp", bufs=4) as outp:
                    for t in range(T):
                        for g in range(2):
                            oft = outp.tile([128, 1024], dt.bfloat16,
                                            tag="of")
                            of = oft[:]
                            src = p16_box[0][:, t * 2048 + g * 1024:
                                             t * 2048 + (g + 1) * 1024]
                            if (t * 2 + g) % 2 == 0:
                                nc.vector.tensor_scalar(
                                    of, src, ap_[:], bp_[:],
                                    op0=Alu.mult, op1=Alu.add)
                            else:
                                nc.scalar.activation(
                                    of, src, Act.Identity,
                                    bias=bp_[:], scale=ap_[:])
                            eng = (nc.sync, nc.scalar)[(t * 2 + g) % 2]
                            eng.dma_start(out_d[t, g], of)
            latestack.close()

    nc.compile()
    return nc


def _host_inputs(x, qw, q_gamma, q_beta, kw, k_gamma, k_beta,
                 v_gamma, v_beta, pw, p_gamma, p_beta):
    """Build the 8 per-core input dicts."""
    f32 = np.float32
    bf16 = ml_dtypes.bfloat16
    f8 = ml_dtypes.float8_e4m3
    x = np.ascontiguousarray(x, f32)
    qw = np.asarray(qw, f32)
    kw = np.asarray(kw, f32)
    pw = np.asarray(pw, f32)

    kd = (qw[:, :, 0] + qw[:, :, 2]).sum((-1, -2))  # [O, I]
    qw_eff = qw.copy()
    qw_eff[:, :, 1, 1, 1] -= THETA * kd
    w27 = qw_eff.reshape(128, 128, 27).transpose(1, 2, 0).reshape(128, 27 * 128)
    w27 = np.ascontiguousarray(w27, f32)
    kwT = np.ascontiguousarray(kw.T, f32)
    pwT = pw.T  # [i, o]
    pw_hi = pwT.astype(bf16)
    pw_lo = (pwT - pw_hi.astype(f32)).astype(bf16)
    pwT2 = np.stack([pw_hi, pw_lo])

    gb = np.stack([q_gamma, q_beta, k_gamma, k_beta, v_gamma, v_beta,
                   p_gamma, p_beta], axis=1).astype(f32)
    ident = (0.25 * np.eye(128)).astype(bf16)
    amats = np.zeros((128, 32), bf16)
    for c in range(128):
        for t in range(T):
            amats[c, t * 8 + c // 16] = 0.5 * (2.0 ** t)
    emat8 = np.zeros((8, 128), f8)
    for c in range(128):
        emat8[c // 16, c] = 1.0

    # x windowed: [t, b, c, wt, dt, wh, dh, ww, dw]
    xw = x.reshape(T, B, C, 2, 4, 2, LH, 2, LW)

    in_maps = []
    for core in range(8):
        b, j = core // 4, core % 4
        xconv = np.zeros((T, 2, 3, 128, 34, 34), f32)
        for g in range(2):
            for dd in range(3):
                d = j + 4 * g + dd - 1
                if 0 <= d < D:
                    xconv[:, g, dd, :, 1:33, 1:33] = x[:, b, :, d]
        # xv[c, w, t, s]: w = wt*4+wh*2+ww, s = dh*16+dw, dt=j
        xvw = xw[:, b, :, :, j]  # [t, c, wt, wh, dh, ww, dw]
        xvw = xvw.transpose(1, 2, 3, 5, 0, 4, 6)  # [c, wt, wh, ww, t, dh, dw]
        xv = xvw.reshape(C, NUM_WINS, T, 256)
        bm16 = np.zeros((128, 16), f32)
        bm16[:, b * 8:(b + 1) * 8] = 1.0
        romats = np.zeros((8, 256), f32)
        for k in range(8):
            for w in range(8):
                for w2 in range(8):
                    if k == w:
                        romats[k, w * 8 + w2] = 1.0
                    if k == w2:
                        romats[k, 64 + w * 8 + w2] = 1.0
        romats[:, 128:256] = 1.0
        in_maps.append({
            "xconv": np.ascontiguousarray(xconv.reshape(T, 2, 3, 128, 34 * 34)),
            "xv": np.ascontiguousarray(xv),
            "w27": w27, "kwT": kwT, "pwT2": pwT2, "gb": gb,
            "identw": ident, "amats": amats, "emat8": emat8, "bmask16": bm16,
            "romats": romats,
        })
    return in_maps


def kernel(**inputs):
    from concourse.bass_utils import run_bass_kernel_spmd

    key = ("dbg" if DEBUG else "plain")
    if key not in _COMPILED:
        _COMPILED[key] = _build(DEBUG)
    nc = _COMPILED[key]

    in_maps = _host_inputs(**inputs)
    res = run_bass_kernel_spmd(nc, in_maps, core_ids=list(range(8)))
    kernel.last_results = res

    full = np.empty((T, B, C, D, H, W), np.float32)
    for core in range(8):
        b, j = core // 4, core % 4
        oc = np.asarray(res.results[core]["out"], dtype=np.float32)
        for g in range(2):
            full[:, b, :, j + 4 * g] = oc[:, g].reshape(T, C, H, W)
    return full


# revision 48
# speedup vs baseline: 1.0280x; 1.0043x over previous
"""Trainium2 Bass kernel for nn_BiSDA (spiking bi-directional sparse attention).

v2 strategy (8 NeuronCores, single SPMD launch), core c = b*4 + j:
  - Layout: all big per-core tensors are [128, 8192] with free index
    w*1024 + t*256 + s  (window-major, s = dh*16+dw within window, dt=j).
  - v-chain (lif(bn(x))) depends only on x: its BN stats AllReduce runs at
    kernel start (also warming the collective path) and the whole v-LIF
    executes on DVE underneath the conv (PE-bound) phase.
  - Conv psum evacuation runs on ACT (Copy + accum_out sum stats); squares
    run as tensor_tensor_reduce on DVE. No DVE copies.
  - Gather-mean over routed windows: fp8e4 DoubleRow matmuls (two stacked
    128-contractions per instruction, 0.5 cyc/row) with mask-scaled
    0.25-identity lhs; spikes are written directly as fp8 (exact).
  - Attention dot: amat_t lhs columns carry 0.5*2^t so the attention LIF
    runs in scaled form (U_t = 2^t u_t), bit-exact in fp16; the z LIF is
    likewise scaled and exact in bf16.
  - q/k LIF stay fp32: q on DVE, k u/W-updates on Pool (k spike on DVE).
  - Final projection: exact bf16 hi/lo split of pw against binary spikes;
    stats via ACT accum; second AllReduce; affine + streamed output DMA.
"""

import os
import sys

import numpy as np

sys.path.insert(0, "/opt/trn_rl_repo")

import ml_dtypes  # noqa: E402

T, B, C = 4, 2, 128
D, H, W = 8, 32, 32
NUM_WINS = 8
LH, LW = 16, 16
NUM_HEADS, HEAD_DIM = 8, 16
THETA = 0.7
EPS = 1e-5
NTOT = float(T * B * D * H * W)
REGION_N = float(T * 4 * LH * LW)

DEBUG = bool(int(os.environ.get("BISDA_DEBUG", "0")))

_COMPILED = {}


def _build(debug):
    import concourse.bacc as bacc
    import concourse.mybir as mybir
    from concourse import tile

    dt = mybir.dt
    Alu = mybir.AluOpType
    Act = mybir.ActivationFunctionType
    DR = mybir.MatmulPerfMode.DoubleRow

    nc = bacc.Bacc("TRN2", target_bir_lowering=False, debug=False,
                   enable_asserts=False, num_devices=8)

    # ---------------- DRAM I/O ----------------
    xconv = nc.dram_tensor("xconv", [T, 2, 3, 128, 34 * 34], dt.float32r,
                           kind="ExternalInput")
    xv = nc.dram_tensor("xv", [128, NUM_WINS, T, 256], dt.float32,
                        kind="ExternalInput")
    w27 = nc.dram_tensor("w27", [128, 27 * 128], dt.float32r, kind="ExternalInput")
    kwT = nc.dram_tensor("kwT", [128, 128], dt.float32r, kind="ExternalInput")
    pwT2 = nc.dram_tensor("pwT2", [2, 128, 128], dt.bfloat16, kind="ExternalInput")
    # gb columns: q_gamma,q_beta,k_gamma,k_beta,v_gamma,v_beta,p_gamma,p_beta
    gb = nc.dram_tensor("gb", [128, 8], dt.float32, kind="ExternalInput")
    identw = nc.dram_tensor("identw", [128, 128], dt.bfloat16,
                            kind="ExternalInput")  # 0.25 * I
    amats = nc.dram_tensor("amats", [128, 32], dt.bfloat16,
                           kind="ExternalInput")  # col t*8+h: 0.5*2^t one-hot
    emat8 = nc.dram_tensor("emat8", [8, 128], dt.float8e4,
                           kind="ExternalInput")  # 1.0 one-hot expand
    bmask16 = nc.dram_tensor("bmask16", [128, 16], dt.float32,
                             kind="ExternalInput")
    # routing broadcast consts: cols 0:64 wsel[k,(w,w')]= (k==w);
    # 64:128 mask1[k,(w,w')] = (k==w'); 128:256 ones
    romats = nc.dram_tensor("romats", [8, 256], dt.float32,
                            kind="ExternalInput")

    out_d = nc.dram_tensor("out", [T, 2, 128, 1024], dt.bfloat16,
                           kind="ExternalOutput")
    dbg = {}
    if debug:
        dbg["qlin"] = nc.dram_tensor("dbg_qlin", [128, 8192], dt.float32,
                                     kind="ExternalOutput")
        dbg["klin"] = nc.dram_tensor("dbg_klin", [128, 8192], dt.float32,
                                     kind="ExternalOutput")
        dbg["stats"] = nc.dram_tensor("dbg_stats", [128, 40], dt.float32,
                                      kind="ExternalOutput")
        dbg["m"] = nc.dram_tensor("dbg_m", [8, 8], dt.float32,
                                  kind="ExternalOutput")
        dbg["qs"] = nc.dram_tensor("dbg_qs", [128, 8192], dt.float8e4,
                                   kind="ExternalOutput")
        dbg["ks"] = nc.dram_tensor("dbg_ks", [128, 8192], dt.float8e4,
                                   kind="ExternalOutput")
        dbg["vs"] = nc.dram_tensor("dbg_vs", [128, 8192], dt.float8e4,
                                   kind="ExternalOutput")
        dbg["attn"] = nc.dram_tensor("dbg_attn", [8, 8192], dt.float16,
                                     kind="ExternalOutput")
        dbg["z"] = nc.dram_tensor("dbg_z", [4, 128, 2048], dt.bfloat16,
                                  kind="ExternalOutput")
        dbg["p"] = nc.dram_tensor("dbg_p", [128, 8192], dt.bfloat16,
                                  kind="ExternalOutput")
        dbg["vag"] = nc.dram_tensor("dbg_vag", [128, 8192], dt.float8e4,
                                    kind="ExternalOutput")
        dbg["qk"] = nc.dram_tensor("dbg_qk", [128, 512], dt.bfloat16,
                                   kind="ExternalOutput")
        dbg["scid"] = nc.dram_tensor("dbg_scid", [128, 8192], dt.float8e4,
                                     kind="ExternalOutput")
        dbg["scid_e"] = nc.dram_tensor("dbg_scid_e", [128, 8192], dt.float8e4,
                                       kind="ExternalOutput")
        dbg["vag_e"] = nc.dram_tensor("dbg_vag_e", [128, 8, 2, 256],
                                      dt.float8e4, kind="ExternalOutput")
        dbg["attn_e"] = nc.dram_tensor("dbg_attn_e", [8, 4096], dt.float16,
                                       kind="ExternalOutput")

    RG = [[0, 1, 2, 3, 4, 5, 6, 7]]
    P2 = [1.0, 2.0, 4.0, 8.0]  # 2^t

    with tile.TileContext(nc) as tc:
        with (
            tc.tile_pool(name="const", bufs=1) as constp,
            tc.tile_pool(name="dram", bufs=1, space="DRAM") as dramp,
            tc.tile_pool(name="big", bufs=1) as bigp,
            tc.tile_pool(name="small", bufs=1) as smallp,
            tc.tile_pool(name="scr512", bufs=2) as scr512,
        ):
            # ---- constants ----
            kwT_sb = constp.tile([128, 128], dt.float32r, tag="kwT")
            nc.sync.dma_start(kwT_sb[:], kwT[:])
            pwT_sb = constp.tile([128, 256], dt.bfloat16, tag="pwT")
            nc.sync.dma_start(pwT_sb[:, 0:128], pwT2[0])
            nc.sync.dma_start(pwT_sb[:, 128:256], pwT2[1])
            gb_sb = constp.tile([128, 8], dt.float32, tag="gb")
            nc.sync.dma_start(gb_sb[:], gb[:])
            ident_sb = constp.tile([128, 128], dt.bfloat16, tag="ident")
            nc.sync.dma_start(ident_sb[:], identw[:])
            amats_sb = constp.tile([128, 32], dt.bfloat16, tag="amats")
            nc.sync.dma_start(amats_sb[:], amats[:])
            emat_sb = constp.tile([8, 128], dt.float8e4, tag="emat")
            nc.sync.dma_start(emat_sb[:], emat8[:])
            bm16_sb = constp.tile([128, 16], dt.float32, tag="bm16")
            nc.sync.dma_start(bm16_sb[:], bmask16[:])
            ro_sb = constp.tile([8, 256], dt.float32, tag="romats")
            nc.sync.dma_start(ro_sb[:], romats[:])

            # persistent big tensors
            qlin = bigp.tile([128, 8192], dt.float32, tag="qlin")
            klin = bigp.tile([128, 8192], dt.float32, tag="klin")
            vs8 = bigp.tile([128, 8192], dt.float8e4, tag="vs8")

            def wts(big):
                return big[:].rearrange("p (w t s) -> p w t s",
                                        w=8, t=4, s=256)

            def v3(tl):
                return tl[:].rearrange("p (w s) -> p w s", w=8, s=256)

            sums = smallp.tile([128, 72], dt.float32, tag="sums")
            # qsum 0:16, qsq 16:32, ksum 32:48, ksq 48:64, xsum 64:66,
            # xsq 66:68  (q/k slots: (t*2+g)*2+hh)
            qregs = smallp.tile([128, 32], dt.float32, tag="qregs")
            kregs = smallp.tile([128, 32], dt.float32, tag="kregs")
            psums = smallp.tile([128, 16], dt.float32, tag="psums")
            psq = smallp.tile([128, 16], dt.float32, tag="psq")
            aff = smallp.tile([128, 16], dt.float32, tag="aff")
            # aff cols: 0 aq_h,1 bq_h,2 ak_h,3 bk_h,4 av_h,5 bv_h,
            #           6 aq_f,7 bq_f,8 ak_f,9 bk_f

            def bn_affine(scol, gcol, St_tile, dst_half, dst_full, name):
                mean = smallp.tile([128, 1], dt.float32, tag=f"mean{name}")
                nc.vector.tensor_scalar(mean[:], St_tile[:, scol:scol + 1],
                                        1.0 / NTOT, None, op0=Alu.mult)
                var = smallp.tile([128, 1], dt.float32, tag=f"var{name}")
                nc.vector.tensor_scalar(var[:], St_tile[:, scol + 1:scol + 2],
                                        1.0 / NTOT, None, op0=Alu.mult)
                msq = smallp.tile([128, 1], dt.float32, tag=f"msq{name}")
                nc.vector.tensor_tensor(msq[:], mean[:], mean[:], op=Alu.mult)
                nc.vector.tensor_tensor(var[:], var[:], msq[:], op=Alu.subtract)
                nc.vector.tensor_scalar(var[:], var[:], EPS, None, op0=Alu.add)
                nc.scalar.sqrt(var[:], var[:])
                rstd = smallp.tile([128, 1], dt.float32, tag=f"rstd{name}")
                nc.vector.reciprocal(rstd[:], var[:])
                afull = smallp.tile([128, 1], dt.float32, tag=f"af{name}")
                nc.vector.tensor_tensor(afull[:], gb_sb[:, gcol:gcol + 1],
                                        rstd[:], op=Alu.mult)
                bfull = smallp.tile([128, 1], dt.float32, tag=f"bf{name}")
                nc.vector.tensor_tensor(bfull[:], afull[:], mean[:], op=Alu.mult)
                nc.vector.tensor_tensor(bfull[:], gb_sb[:, gcol + 1:gcol + 2],
                                        bfull[:], op=Alu.subtract)
                nc.vector.tensor_scalar(aff[:, dst_half:dst_half + 1], afull[:],
                                        0.5, None, op0=Alu.mult)
                nc.vector.tensor_scalar(aff[:, dst_half + 1:dst_half + 2],
                                        bfull[:], 0.5, None, op0=Alu.mult)
                if dst_full is not None:
                    nc.vector.tensor_copy(aff[:, dst_full:dst_full + 1], afull[:])
                    nc.vector.tensor_copy(aff[:, dst_full + 1:dst_full + 2],
                                          bfull[:])

            # ============ STAGE 0 + A: xv, x-stats, AR_v, conv ============
            with (
                tc.tile_pool(name="xvp", bufs=1) as xvp,
                tc.tile_pool(name="vlif", bufs=3) as vlifp,
                tc.tile_pool(name="xin", bufs=6) as xinp,
                tc.tile_pool(name="psA", bufs=6, space="PSUM") as psA,
                tc.tile_pool(name="psK", bufs=2, space="PSUM") as psK,
                tc.tile_pool(name="w27p", bufs=1) as w27p,
            ):
                w27_sb = w27p.tile([128, 27 * 128], dt.float32r, tag="w27")
                nc.sync.dma_start(w27_sb[:, 0:9 * 128], w27[:, 0:9 * 128])
                xpads0 = {}
                for dd in range(3):
                    for g in range(2):
                        xp = xinp.tile([128, 34 * 34], dt.float32r, tag="xpad")
                        nc.sync.dma_start(xp[:], xconv[0, g, dd])
                        xpads0[(g, dd)] = xp
                nc.sync.dma_start(w27_sb[:, 9 * 128:27 * 128],
                                  w27[:, 9 * 128:27 * 128])
                xv_sb = xvp.tile([128, 8192], dt.float32, tag="xv")

                # ---- conv ----
                for t in range(T):
                    if t == 3:
                        nc.sync.dma_start(
                            xv_sb[:],
                            xv[:].rearrange("p w t s -> p (w t s)"))
                        for q4 in range(4):
                            xsl = xv_sb[:, q4 * 2048:(q4 + 1) * 2048]
                            xsc = vlifp.tile([128, 2048], dt.float32,
                                             tag="vu")
                            nc.scalar.activation(
                                xsc[:], xsl, Act.Copy,
                                accum_out=sums[:, 64 + q4:65 + q4])
                            xsc2 = vlifp.tile([128, 2048], dt.float32,
                                              tag="vu")
                            nc.scalar.activation(
                                xsc2[:], xsl, Act.Square,
                                accum_out=sums[:, 68 + q4:69 + q4])
                        pay_v = smallp.tile([128, 8], dt.float32, tag="pay_v")
                        nc.gpsimd.memset(pay_v[:], 0.0)
                        nc.vector.tensor_reduce(
                            pay_v[:, 0:1], sums[:, 64:68],
                            axis=mybir.AxisListType.X, op=Alu.add)
                        nc.vector.tensor_reduce(
                            pay_v[:, 1:2], sums[:, 68:72],
                            axis=mybir.AxisListType.X, op=Alu.add)
                        ccv_in = dramp.tile([128, 8], dt.float32,
                                            tag="ccv_in")
                        ccv_out = dramp.tile([128, 8], dt.float32,
                                             tag="ccv_out")
                        nc.sync.dma_start(ccv_in[:], pay_v[:])
                        nc.gpsimd.collective_compute(
                            "AllReduce", Alu.add, replica_groups=RG,
                            ins=[ccv_in.opt()], outs=[ccv_out.opt()])
                        Sv = smallp.tile([128, 8], dt.float32, tag="Sv")
                        nc.sync.dma_start(Sv[:], ccv_out[:])
                        bn_affine(0, 4, Sv, 4, None, "v")
                    if t == 0:
                        xpads = xpads0
                    else:
                        xpads = {}
                        for dd in range(3):
                            for g in range(2):
                                xp = xinp.tile([128, 34 * 34], dt.float32r,
                                               tag="xpad")
                                nc.sync.dma_start(xp[:], xconv[t, g, dd])
                                xpads[(g, dd)] = xp
                    pss = {}
                    for g in range(2):
                        for hh in range(2):
                            psx = psA.tile([128, 512], dt.float32, tag="psA")
                            pss[(g, hh)] = psx
                    for tap in range(27):
                        dd, rem = divmod(tap, 9)
                        dhh, dww = divmod(rem, 3)
                        lhs = w27_sb[:, tap * 128:(tap + 1) * 128]
                        for g in range(2):
                            for hh in range(2):
                                rhs = xpads[(g, dd)][:].rearrange(
                                    "p (r c) -> p r c", r=34, c=34
                                )[:, 16 * hh + dhh:16 * hh + dhh + 16,
                                  dww:dww + 32]
                                nc.tensor.matmul(pss[(g, hh)][:], lhs, rhs,
                                                 start=(tap == 0),
                                                 stop=(tap == 26))
                    # k_lin from center planes, rhs (ww, dh, dw)
                    for g in range(2):
                        for hh in range(2):
                            slot = (t * 2 + g) * 2 + hh
                            psk = psK.tile([128, 512], dt.float32, tag="psK")
                            ctr = xpads[(g, 1)][:].rearrange(
                                "p (r c) -> p r c", r=34, c=34)
                            rhs = ctr[:, 16 * hh + 1:16 * hh + 17, 1:33]
                            rhs = rhs.rearrange(
                                "p dh (ww dw) -> p ww dh dw", ww=2, dw=16)
                            nc.tensor.matmul(psk[:], kwT_sb[:], rhs,
                                             start=True, stop=True)
                            w0 = 4 * g + 2 * hh
                            kdst = wts(klin)[:, w0:w0 + 2, t]
                            nc.scalar.activation(
                                kdst,
                                psk[:].rearrange("p (ww s) -> p ww s",
                                                 ww=2, s=256),
                                Act.Copy,
                                accum_out=sums[:, 32 + slot:33 + slot])
                            ksc = scr512.tile([128, 512], dt.float32,
                                              tag="sq512")
                            nc.scalar.activation(
                                ksc[:], psk[:], Act.Square,
                                accum_out=sums[:, 48 + slot:49 + slot])
                        rc = g * 16 + t * 4
                        nc.vector.tensor_reduce(
                            kregs[:, rc:rc + 4],
                            wts(klin)[:, 4 * g:4 * g + 4, t],
                            axis=mybir.AxisListType.X, op=Alu.add)
                    for g in range(2):
                        for hh in range(2):
                            slot = (t * 2 + g) * 2 + hh
                            w0 = 4 * g + 2 * hh
                            dstap = wts(qlin)[:, w0:w0 + 2, t].rearrange(
                                "p ww (dh dw) -> p dh ww dw", dh=16, dw=16)
                            nc.scalar.activation(
                                dstap, pss[(g, hh)][:].rearrange(
                                    "p (dh ww dw) -> p dh ww dw",
                                    ww=2, dh=16, dw=16),
                                Act.Copy,
                                accum_out=sums[:, slot:slot + 1])
                            qsc = scr512.tile([128, 512], dt.float32,
                                              tag="sq512")
                            nc.scalar.activation(
                                qsc[:], pss[(g, hh)][:], Act.Square,
                                accum_out=sums[:, 16 + slot:17 + slot])
                        rc = g * 16 + t * 4
                        nc.vector.tensor_reduce(
                            qregs[:, rc:rc + 4],
                            wts(qlin)[:, 4 * g:4 * g + 4, t],
                            axis=mybir.AxisListType.X, op=Alu.add)

                    if t == 3:
                        wv = None
                        for tv in range(T):
                            yv = vlifp.tile([128, 2048], dt.float32, tag="vu")
                            nc.scalar.activation(
                                v3(yv), wts(xv_sb)[:, :, tv],
                                Act.Identity, bias=aff[:, 5:6],
                                scale=aff[:, 4:5])
                            if tv == 0:
                                u = yv
                            else:
                                un = vlifp.tile([128, 2048], dt.float32,
                                                tag="vu")
                                nc.vector.scalar_tensor_tensor(
                                    un[:], wv[:], 0.5, yv[:],
                                    op0=Alu.mult, op1=Alu.add)
                                u = un
                            nc.vector.tensor_scalar(
                                wts(vs8)[:, :, tv], v3(u), 1.0, None,
                                op0=Alu.is_ge)
                            if tv < T - 1:
                                wn = vlifp.tile([128, 2048], dt.float32,
                                                tag="vu")
                                nc.vector.scalar_tensor_tensor(
                                    wn[:], u[:], 1.0, u[:],
                                    op0=Alu.is_lt, op1=Alu.mult)
                                wv = wn

            # ============ STAGE B: AR1 + affines + routing ============
            from contextlib import ExitStack
            latestack = ExitStack()
            latep = latestack.enter_context(tc.tile_pool(name="late", bufs=1))
            p16g = latep.tile([128, 8192], dt.bfloat16, tag="p16")
            qs8 = latep.tile([128, 8192], dt.float8e4, tag="qs8")
            ks8 = latep.tile([128, 8192], dt.float8e4, tag="ks8")
            scid8 = latep.tile([128, 8192], dt.float8e4, tag="scid8")
            vag8 = latep.tile([128, 8192], dt.float8e4, tag="vag8")

            payload = smallp.tile([128, 40], dt.float32, tag="payload")
            for col, (base, cnt) in enumerate(
                    [(0, 16), (16, 16), (32, 16), (48, 16)]):
                nc.vector.tensor_reduce(
                    payload[:, col:col + 1], sums[:, base:base + cnt],
                    axis=mybir.AxisListType.X, op=Alu.add)
            qreg8 = smallp.tile([128, 8], dt.float32, tag="qreg8")
            nc.vector.tensor_reduce(
                qreg8[:].rearrange("p (g w) -> p g w", g=2, w=4),
                qregs[:].rearrange("p (g t w) -> p g w t", g=2, t=4, w=4),
                axis=mybir.AxisListType.X, op=Alu.add)
            kreg8 = smallp.tile([128, 8], dt.float32, tag="kreg8")
            nc.vector.tensor_reduce(
                kreg8[:].rearrange("p (g w) -> p g w", g=2, w=4),
                kregs[:].rearrange("p (g t w) -> p g w t", g=2, t=4, w=4),
                axis=mybir.AxisListType.X, op=Alu.add)
            nc.vector.tensor_tensor(payload[:, 4:12], qreg8[:],
                                    bm16_sb[:, 0:8], op=Alu.mult)
            nc.vector.tensor_tensor(payload[:, 12:20], qreg8[:],
                                    bm16_sb[:, 8:16], op=Alu.mult)
            nc.vector.tensor_tensor(payload[:, 20:28], kreg8[:],
                                    bm16_sb[:, 0:8], op=Alu.mult)
            nc.vector.tensor_tensor(payload[:, 28:36], kreg8[:],
                                    bm16_sb[:, 8:16], op=Alu.mult)
            nc.gpsimd.memset(payload[:, 36:40], 0.0)

            cc1_in = dramp.tile([128, 40], dt.float32, tag="cc1_in")
            cc1_out = dramp.tile([128, 40], dt.float32, tag="cc1_out")
            nc.sync.dma_start(cc1_in[:], payload[:])
            nc.gpsimd.collective_compute(
                "AllReduce", Alu.add, replica_groups=RG,
                ins=[cc1_in.opt()], outs=[cc1_out.opt()])
            St = smallp.tile([128, 40], dt.float32, tag="St")
            nc.sync.dma_start(St[:], cc1_out[:])
            if debug:
                nc.sync.dma_start(dbg["stats"][:], St[:])

            bn_affine(0, 0, St, 0, 6, "q")
            bn_affine(2, 2, St, 2, 8, "k")
            aff2 = smallp.tile([128, 16], dt.float32, tag="aff2")
            for tt in range(4):
                for (cc, base) in ((0, 0), (2, 8)):
                    nc.vector.tensor_scalar(
                        aff2[:, base + 2 * tt:base + 2 * tt + 2],
                        aff[:, cc:cc + 2], float(2.0 ** tt), None,
                        op0=Alu.mult)

            # routing: region means -> BN -> select b -> a_r -> top4 mask
            qr16 = smallp.tile([128, 16], dt.float32, tag="qr16")
            nc.vector.tensor_scalar(qr16[:], St[:, 4:20], 1.0 / REGION_N, None,
                                    op0=Alu.mult)
            nc.scalar.activation(qr16[:], qr16[:], Act.Identity,
                                 bias=aff[:, 7:8], scale=aff[:, 6:7])
            kr16 = smallp.tile([128, 16], dt.float32, tag="kr16")
            nc.vector.tensor_scalar(kr16[:], St[:, 20:36], 1.0 / REGION_N, None,
                                    op0=Alu.mult)
            nc.scalar.activation(kr16[:], kr16[:], Act.Identity,
                                 bias=aff[:, 9:10], scale=aff[:, 8:9])
            tmp8 = smallp.tile([128, 8], dt.float32, tag="tmp8")
            nc.vector.scalar_tensor_tensor(
                tmp8[:], qr16[:, 0:8], bm16_sb[:, 0:1], qr16[:, 8:16],
                op0=Alu.mult, op1=Alu.bypass)
            nc.vector.scalar_tensor_tensor(
                tmp8[:], qr16[:, 8:16], bm16_sb[:, 8:9], tmp8[:],
                op0=Alu.mult, op1=Alu.add)
            tmpk8 = smallp.tile([128, 8], dt.float32, tag="tmpk8")
            nc.vector.scalar_tensor_tensor(
                tmpk8[:], kr16[:, 0:8], bm16_sb[:, 0:1], kr16[:, 8:16],
                op0=Alu.mult, op1=Alu.bypass)
            nc.vector.scalar_tensor_tensor(
                tmpk8[:], kr16[:, 8:16], bm16_sb[:, 8:9], tmpk8[:],
                op0=Alu.mult, op1=Alu.add)
            with tc.tile_pool(name="psB", bufs=1, space="PSUM") as psB:
                ar_ps = psB.tile([8, 8], dt.float32, tag="ar")
                nc.tensor.matmul(ar_ps[:], tmp8[:], tmpk8[:], start=True,
                                 stop=True)
                ar = smallp.tile([8, 8], dt.float32, tag="arsb")
                nc.vector.tensor_copy(ar[:], ar_ps[:])
            srt = smallp.tile([8, 8], dt.float32, tag="srt")
            nc.vector.max(srt[:], ar[:])
            msel = smallp.tile([8, 8], dt.float32, tag="msel")
            nc.vector.tensor_scalar(msel[:], ar[:], srt[:, 3:4], None,
                                    op0=Alu.is_ge)
            if debug:
                nc.sync.dma_start(dbg["m"][:], msel[:])
            with tc.tile_pool(name="psB2", bufs=1, space="PSUM") as psB2:
                o1 = psB2.tile([8, 64], dt.float32, tag="o1")
                nc.tensor.matmul(o1[:], msel[:], ro_sb[:, 0:64],
                                 start=True, stop=True)
                o1m = smallp.tile([8, 64], dt.float32, tag="o1m")
                nc.vector.tensor_tensor(o1m[:], o1[:], ro_sb[:, 64:128],
                                        op=Alu.mult)
                mb_ps = psB2.tile([128, 64], dt.float32, tag="mbps")
                nc.tensor.matmul(mb_ps[:], ro_sb[:, 128:256], o1m[:],
                                 start=True, stop=True)
                mbc = smallp.tile([128, 64], dt.float32, tag="mbc")
                nc.vector.tensor_copy(mbc[:], mb_ps[:])
            # scid8[c, (w*8+w')*128 + c'] = 0.25*mask (ident_sb = 0.25*I)
            for w in range(8):
                for w2 in range(8):
                    i = w * 8 + w2
                    if w % 2 == 0:
                        nc.vector.tensor_scalar(
                            scid8[:, i * 128:(i + 1) * 128], ident_sb[:],
                            mbc[:, i:i + 1], None, op0=Alu.mult)
                    else:
                        nc.scalar.activation(
                            scid8[:, i * 128:(i + 1) * 128], ident_sb[:],
                            Act.Identity, scale=mbc[:, i:i + 1])
            tc.strict_bb_all_engine_barrier()
            if debug:
                nc.sync.dma_start(dbg["scid_e"][:], scid8[:])

            # ============ STAGE C: pipelined LIF + gather + attn + out ======
            attn_half = {}
            uq_prev = [None]
            uk_prev = [None]
            uat_prev = [None]
            uz_prev = [None]
            p16_box = [None]

            from contextlib import ExitStack
            with ExitStack() as cstack:
                psKV = cstack.enter_context(
                    tc.tile_pool(name="psKV", bufs=2, space="PSUM"))
                psAt = cstack.enter_context(
                    tc.tile_pool(name="psAt", bufs=1, space="PSUM"))
                psEx = cstack.enter_context(
                    tc.tile_pool(name="psEx", bufs=1, space="PSUM"))
                psP = cstack.enter_context(
                    tc.tile_pool(name="psP", bufs=2, space="PSUM"))
                athfp = cstack.enter_context(tc.tile_pool(name="athf", bufs=1))
                atup = cstack.enter_context(tc.tile_pool(name="atuw", bufs=3))
                atwp = atup
                atsp = cstack.enter_context(tc.tile_pool(name="ats", bufs=2))
                qkp = cstack.enter_context(tc.tile_pool(name="qkp", bufs=2))
                yzp = cstack.enter_context(tc.tile_pool(name="zscr", bufs=3))
                zup = yzp
                zwp = yzp
                ztp = cstack.enter_context(tc.tile_pool(name="ztp", bufs=2))
                qlifp = cstack.enter_context(tc.tile_pool(name="lif", bufs=6))
                klifp = qlifp
                def lif_qk(t):
                    # scaled fp16 LIF: U_t = 2^t u_t = W_{t-1} + Y_t,
                    # Y_t = 2^t*(a_h*lin + b_h); spike U>=2^t; W = U*[U<2^t]
                    for (lin, spk, base, upool, upr) in (
                            (qlin, qs8, 0, qlifp, uq_prev),
                            (klin, ks8, 8, klifp, uk_prev)):
                        ysl = wts(lin)[:, :, t]
                        yt = upool.tile([128, 2048], dt.float16, tag="qu")
                        nc.scalar.activation(
                            v3(yt), ysl, Act.Identity,
                            bias=aff2[:, base + 2 * t + 1:base + 2 * t + 2],
                            scale=aff2[:, base + 2 * t:base + 2 * t + 1])
                        if t == 0:
                            uap = yt[:]
                        else:
                            ut = upool.tile([128, 2048], dt.float16, tag="qu")
                            nc.vector.tensor_tensor(ut[:], upr[0], yt[:],
                                                    op=Alu.add)
                            uap = ut[:]
                        nc.vector.tensor_scalar(
                            wts(spk)[:, :, t],
                            uap.rearrange("p (w s) -> p w s", w=8, s=256),
                            float(2.0 ** t), None, op0=Alu.is_ge)
                        if t < T - 1:
                            mk = upool.tile([128, 2048], dt.float16, tag="qu")
                            nc.vector.tensor_scalar(
                                mk[:], uap, float(2.0 ** t), None,
                                op0=Alu.is_lt)
                            wt_ = upool.tile([128, 2048], dt.float16,
                                             tag="qu")
                            nc.vector.tensor_tensor(wt_[:], uap, mk[:],
                                                    op=Alu.mult)
                            upr[0] = wt_[:]

                def gather_half(half):
                    at_sb = athfp.tile([8, 4096], dt.float16, tag="athf")
                    attn_half[half] = at_sb
                    ks8v = ks8[:].rearrange("p (w hs) -> p w hs", w=8, hs=1024)
                    vs8v = vs8[:].rearrange("p (w hs) -> p w hs", w=8, hs=1024)
                    sc8v = scid8[:].rearrange("p (b c) -> p b c", b=64, c=128)
                    hs = slice(half * 512, half * 512 + 512)
                    for w in range(NUM_WINS):
                        kag = psKV.tile([128, 512], dt.float32, tag="kag")
                        vag = psKV.tile([128, 512], dt.float32, tag="vag")
                        for ps, src in ((kag, ks8v), (vag, vs8v)):
                            for pr in range(4):
                                lhs = sc8v[:, w * 8 + 2 * pr:
                                           w * 8 + 2 * pr + 2]
                                rhs = src[:, 2 * pr:2 * pr + 2, hs]
                                nc.tensor.matmul(ps[:], lhs, rhs,
                                                 start=(pr == 0),
                                                 stop=(pr == 3),
                                                 perf_mode=DR)
                        nc.scalar.activation(
                            vag8[:, w * 1024 + half * 512:
                                 w * 1024 + half * 512 + 512],
                            vag[:], Act.Copy)
                        qk = qkp.tile([128, 512], dt.bfloat16, tag="qk")
                        nc.vector.tensor_tensor(
                            qk[:], qs8[:, w * 1024 + half * 512:
                                        w * 1024 + half * 512 + 512],
                            kag[:], op=Alu.mult)
                        if debug and half == 0 and w == 0:
                            nc.sync.dma_start(dbg["qk"][:], qk[:])
                        at = psAt.tile([8, 512], dt.float32, tag="at")
                        for ts_ in range(2):
                            t = 2 * half + ts_
                            nc.tensor.matmul(
                                at[:, ts_ * 256:(ts_ + 1) * 256],
                                amats_sb[:, t * 8:t * 8 + 8],
                                qk[:, ts_ * 256:(ts_ + 1) * 256],
                                start=True, stop=True)
                        nc.scalar.activation(
                            at_sb[:, w * 512:(w + 1) * 512], at[:], Act.Copy)

                def attn_lif(t):
                    half, ts_ = t // 2, t % 2
                    ysl = attn_half[half][:].rearrange(
                        "p (w u s) -> p w u s", w=8, u=2, s=256)[:, :, ts_]
                    if t == 0:
                        uap = ysl
                    else:
                        ut = atup.tile([8, 2048], dt.float16, tag="atu")
                        nc.vector.tensor_tensor(
                            ut[:].rearrange("p (w s) -> p w s", w=8, s=256),
                            uat_prev[0], ysl, op=Alu.add)
                        uap = ut[:].rearrange("p (w s) -> p w s", w=8, s=256)
                    at_s = atsp.tile([8, 2048], dt.float8e4, tag="ats")
                    nc.vector.tensor_scalar(
                        at_s[:].rearrange("p (w s) -> p w s", w=8, s=256),
                        uap, P2[t], None, op0=Alu.is_ge)
                    if t < T - 1:
                        wt_ = atwp.tile([8, 2048], dt.float16, tag="atu")
                        nc.vector.scalar_tensor_tensor(
                            wt_[:].rearrange("p (w s) -> p w s", w=8, s=256),
                            uap, P2[t], uap, op0=Alu.is_lt, op1=Alu.mult)
                        uat_prev[0] = wt_[:].rearrange(
                            "p (w s) -> p w s", w=8, s=256)
                    return at_s

                def z_stage(t, at_s):
                    p16 = p16_box[0]
                    yz = yzp.tile([128, 2048], dt.bfloat16, tag="yz")
                    at_v = at_s[:].rearrange("p (w s) -> p w s", w=8, s=256)
                    for wp in range(4):
                        ex = psEx.tile([128, 512], dt.float32, tag="ex")
                        nc.tensor.matmul(ex[:], emat_sb[:],
                                         at_v[:, 2 * wp:2 * wp + 2],
                                         start=True, stop=True)
                        vsl = wts(vag8)[:, 2 * wp:2 * wp + 2, t]
                        ydst = yz[:, wp * 512:(wp + 1) * 512].rearrange(
                            "p (w s) -> p w s", w=2, s=256)
                        nc.vector.scalar_tensor_tensor(
                            ydst, ex[:].rearrange("p (w s) -> p w s",
                                                  w=2, s=256),
                            0.5 * P2[t], vsl, op0=Alu.mult, op1=Alu.mult)
                    if t == 0:
                        uap = yz[:]
                    else:
                        ut = zup.tile([128, 2048], dt.bfloat16, tag="yz")
                        nc.vector.tensor_tensor(ut[:], uz_prev[0], yz[:],
                                                op=Alu.add)
                        uap = ut[:]
                    zt = ztp.tile([128, 2048], dt.bfloat16, tag="zt")
                    nc.vector.tensor_scalar(zt[:], uap, P2[t], None,
                                            op0=Alu.is_ge)
                    if t < T - 1:
                        mkz = zwp.tile([128, 2048], dt.bfloat16, tag="yz")
                        nc.vector.tensor_scalar(mkz[:], uap, P2[t], None,
                                                op0=Alu.is_lt)
                        wt_ = zwp.tile([128, 2048], dt.bfloat16, tag="yz")
                        nc.vector.tensor_tensor(wt_[:], uap, mkz[:],
                                                op=Alu.mult)
                        uz_prev[0] = wt_[:]
                    if debug:
                        nc.sync.dma_start(dbg["z"][t], zt[:])
                    ztv = zt[:].rearrange("p (w dh dw) -> p w dh dw",
                                          w=8, dh=16, dw=16)
                    for g in range(2):
                        for hh in range(2):
                            w0 = 4 * g + 2 * hh
                            rhs = ztv[:, w0:w0 + 2].rearrange(
                                "p ww dh dw -> p dh ww dw")
                            pp = psP.tile([128, 512], dt.float32, tag="pp")
                            nc.tensor.matmul(pp[:], pwT_sb[:, 0:128], rhs,
                                             start=True, stop=True)
                            slot = (t * 2 + g) * 2 + hh
                            dst = p16[:, t * 2048 + g * 1024 + hh * 512:
                                      t * 2048 + g * 1024 + hh * 512 + 512]
                            nc.scalar.activation(
                                dst, pp[:], Act.Copy,
                                accum_out=psums[:, slot:slot + 1])
                            pscr = scr512.tile([128, 512], dt.float32,
                                               tag="sq512")
                            nc.scalar.activation(
                                pscr[:], pp[:], Act.Square,
                                accum_out=psq[:, slot:slot + 1])

                lif_qk(0)
                lif_qk(1)
                gather_half(0)
                if debug:
                    nc.sync.dma_start(dbg["vag_e"][:],
                                      wts(vag8)[:, :, 0:2])
                    nc.sync.dma_start(dbg["attn_e"][:], attn_half[0][:])
                lif_qk(2)
                lif_qk(3)
                if debug:
                    nc.sync.dma_start(dbg["qlin"][:], qlin[:])
                    nc.sync.dma_start(dbg["klin"][:], klin[:])
                p16_box[0] = p16g
                z_stage(0, attn_lif(0))
                gather_half(1)
                z_stage(1, attn_lif(1))
                for t in (2, 3):
                    z_stage(t, attn_lif(t))

                if debug:
                    nc.sync.dma_start(dbg["vag"][:], vag8[:])
                    nc.sync.dma_start(dbg["scid"][:], scid8[:])
                    nc.sync.dma_start(dbg["qs"][:], qs8[:])
                    nc.sync.dma_start(dbg["ks"][:], ks8[:])
                    nc.sync.dma_start(dbg["vs"][:], vs8[:])
                    nc.sync.dma_start(dbg["attn"][:, 0:4096], attn_half[0][:])
                    nc.sync.dma_start(dbg["attn"][:, 4096:8192],
                                      attn_half[1][:])
                    nc.sync.dma_start(dbg["p"][:], p16_box[0][:])

                # ============ STAGE D: AR2 + final affine + out DMA ========
                pay2 = smallp.tile([128, 8], dt.float32, tag="pay2")
                nc.vector.tensor_reduce(pay2[:, 0:1], psums[:],
                                        axis=mybir.AxisListType.X, op=Alu.add)
                nc.vector.tensor_reduce(pay2[:, 1:2], psq[:],
                                        axis=mybir.AxisListType.X, op=Alu.add)
                nc.gpsimd.memset(pay2[:, 2:8], 0.0)
                cc2_in = dramp.tile([128, 8], dt.float32, tag="cc2_in")
                cc2_out = dramp.tile([128, 8], dt.float32, tag="cc2_out")
                nc.sync.dma_start(cc2_in[:], pay2[:])
                nc.gpsimd.collective_compute(
                    "AllReduce", Alu.add, replica_groups=RG,
                    ins=[cc2_in.opt()], outs=[cc2_out.opt()])
                S2 = smallp.tile([128, 8], dt.float32, tag="S2")
                nc.sync.dma_start(S2[:], cc2_out[:])

                meanp = smallp.tile([128, 1], dt.float32, tag="meanp")
                nc.vector.tensor_scalar(meanp[:], S2[:, 0:1], 1.0 / NTOT,
                                        None, op0=Alu.mult)
                varp = smallp.tile([128, 1], dt.float32, tag="varp")
                nc.vector.tensor_scalar(varp[:], S2[:, 1:2], 1.0 / NTOT,
                                        None, op0=Alu.mult)
                msqp = smallp.tile([128, 1], dt.float32, tag="msqp")
                nc.vector.tensor_tensor(msqp[:], meanp[:], meanp[:],
                                        op=Alu.mult)
                nc.vector.tensor_tensor(varp[:], varp[:], msqp[:],
                                        op=Alu.subtract)
                nc.vector.tensor_scalar(varp[:], varp[:], EPS, None,
                                        op0=Alu.add)
                nc.scalar.sqrt(varp[:], varp[:])
                rstdp = smallp.tile([128, 1], dt.float32, tag="rstdp")
                nc.vector.reciprocal(rstdp[:], varp[:])
                ap_ = smallp.tile([128, 1], dt.float32, tag="ap_")
                nc.vector.tensor_tensor(ap_[:], gb_sb[:, 6:7], rstdp[:],
                                        op=Alu.mult)
                bp_ = smallp.tile([128, 1], dt.float32, tag="bp_")
                nc.vector.tensor_tensor(bp_[:], ap_[:], meanp[:], op=Alu.mult)
                nc.vector.tensor_tensor(bp_[:], gb_sb[:, 7:8], bp_[:],
                                        op=Alu.subtract)

                with tc.tile_pool(name="outp", bufs=4) as outp:
                    for t in range(T):
                        for g in range(2):
                            oft = outp.tile([128, 1024], dt.bfloat16,
                                            tag="of")
                            of = oft[:]
                            src = p16_box[0][:, t * 2048 + g * 1024:
                                             t * 2048 + (g + 1) * 1024]
                            if (t * 2 + g) % 2 == 0:
                                nc.vector.tensor_scalar(
                                    of, src, ap_[:], bp_[:],
                                    op0=Alu.mult, op1=Alu.add)
                            else:
                                nc.scalar.activation(
                                    of, src, Act.Identity,
                                    bias=bp_[:], scale=ap_[:])
                            eng = (nc.sync, nc.scalar)[(t * 2 + g) % 2]
                            eng.dma_start(out_d[t, g], of)
            latestack.close()

    nc.compile()
    return nc


def _host_inputs(x, qw, q_gamma, q_beta, kw, k_gamma, k_beta,
                 v_gamma, v_beta, pw, p_gamma, p_beta):
    """Build the 8 per-core input dicts."""
    f32 = np.float32
    bf16 = ml_dtypes.bfloat16
    f8 = ml_dtypes.float8_e4m3
    x = np.ascontiguousarray(x, f32)
    qw = np.asarray(qw, f32)
    kw = np.asarray(kw, f32)
    pw = np.asarray(pw, f32)

    kd = (qw[:, :, 0] + qw[:, :, 2]).sum((-1, -2))  # [O, I]
    qw_eff = qw.copy()
    qw_eff[:, :, 1, 1, 1] -= THETA * kd
    w27 = qw_eff.reshape(128, 128, 27).transpose(1, 2, 0).reshape(128, 27 * 128)
    w27 = np.ascontiguousarray(w27, f32)
    kwT = np.ascontiguousarray(kw.T, f32)
    pwT = pw.T  # [i, o]
    pw_hi = pwT.astype(bf16)
    pw_lo = (pwT - pw_hi.astype(f32)).astype(bf16)
    pwT2 = np.stack([pw_hi, pw_lo])

    gb = np.stack([q_gamma, q_beta, k_gamma, k_beta, v_gamma, v_beta,
                   p_gamma, p_beta], axis=1).astype(f32)
    ident = (0.25 * np.eye(128)).astype(bf16)
    amats = np.zeros((128, 32), bf16)
    for c in range(128):
        for t in range(T):
            amats[c, t * 8 + c // 16] = 0.5 * (2.0 ** t)
    emat8 = np.zeros((8, 128), f8)
    for c in range(128):
        emat8[c // 16, c] = 1.0

    # x windowed: [t, b, c, wt, dt, wh, dh, ww, dw]
    xw = x.reshape(T, B, C, 2, 4, 2, LH, 2, LW)

    in_maps = []
    for core in range(8):
        b, j = core // 4, core % 4
        xconv = np.zeros((T, 2, 3, 128, 34, 34), f32)
        for g in range(2):
            for dd in range(3):
                d = j + 4 * g + dd - 1
                if 0 <= d < D:
                    xconv[:, g, dd, :, 1:33, 1:33] = x[:, b, :, d]
        # xv[c, w, t, s]: w = wt*4+wh*2+ww, s = dh*16+dw, dt=j
        xvw = xw[:, b, :, :, j]  # [t, c, wt, wh, dh, ww, dw]
        xvw = xvw.transpose(1, 2, 3, 5, 0, 4, 6)  # [c, wt, wh, ww, t, dh, dw]
        xv = xvw.reshape(C, NUM_WINS, T, 256)
        bm16 = np.zeros((128, 16), f32)
        bm16[:, b * 8:(b + 1) * 8] = 1.0
        romats = np.zeros((8, 256), f32)
        for k in range(8):
            for w in range(8):
                for w2 in range(8):
                    if k == w:
                        romats[k, w * 8 + w2] = 1.0
                    if k == w2:
                        romats[k, 64 + w * 8 + w2] = 1.0
        romats[:, 128:256] = 1.0
        in_maps.append({
            "xconv": np.ascontiguousarray(xconv.reshape(T, 2, 3, 128, 34 * 34)),
            "xv": np.ascontiguousarray(xv),
            "w27": w27, "kwT": kwT, "pwT2": pwT2, "gb": gb,
            "identw": ident, "amats": amats, "emat8": emat8, "bmask16": bm16,
            "romats": romats,
        })
    return in_maps


def kernel(**inputs):
    from concourse.bass_utils import run_bass_kernel_spmd

    key = ("dbg" if DEBUG else "plain")
    if key not in _COMPILED:
        _COMPILED[key] = _build(DEBUG)
    nc = _COMPILED[key]

    in_maps = _host_inputs(**inputs)
    res = run_bass_kernel_spmd(nc, in_maps, core_ids=list(range(8)))
    kernel.last_results = res

    full = np.empty((T, B, C, D, H, W), np.float32)
    for core in range(8):
        b, j = core // 4, core % 4
        oc = np.asarray(res.results[core]["out"], dtype=np.float32)
        for g in range(2):
            full[:, b, :, j + 4 * g] = oc[:, g].reshape(T, C, H, W)
    return full
